# revision 1
# baseline (speedup 1.0000x reference)
"""Trainium2 Bass kernel for nn_Attention: 16-head attention layer, B=2, S=2048, H=1024.

Strategy (Megatron-style tensor parallel over heads, 8 cores x 2 heads):
  - Host transposes hidden_states once (XT [H, B*S]) and pre-rounds all matmul
    inputs to fp32r (TF32-like: 11-bit mantissa) so every matmul runs at the
    full 1-cycle/row PE rate with fp32 accumulation.
  - Each core computes its 2 heads' q/k/v via XT @ its W slice (transposed
    layout), attention with softmax folded as exp -> matmul-rowsum -> late
    normalization, then a partial dense projection over its 128 ctx columns.
  - Host sums the 8 partial dense outputs and adds dense_b.

All computed on device except the final 8-way partial reduction (done at
gather time on host, per the Megatron all-reduce-after-dense recipe).
"""
import os
import numpy as np

B, S, H, NH = 2, 2048, 1024, 16
HD = H // NH            # 64
BS = B * S              # 4096
NCORES = 8
ROWS_PER_CORE = 3 * HD * 2   # 384 qkv rows per core
DPC = 2 * HD                 # 128 ctx/dense columns per core

_CACHE = {}


def _round_fp32r(x):
    bits = np.ascontiguousarray(x, dtype=np.float32).view(np.uint32)
    lsb = (bits >> np.uint32(12)) & np.uint32(1)
    return ((bits + np.uint32(0x7FF) + lsb) & np.uint32(0xFFFFF000)).view(np.float32)


def _build_program():
    import concourse.mybir as mybir
    import concourse.tile as tile
    from concourse import bacc

    F32 = mybir.dt.float32
    F32R = mybir.dt.float32r
    Act = mybir.ActivationFunctionType

    nc = bacc.Bacc("TRN2", target_bir_lowering=False, debug=False,
                   num_devices=NCORES)
    xt = nc.dram_tensor("xt", [H, BS], F32R, kind="ExternalInput").ap()
    w1t = nc.dram_tensor("w1t", [H, ROWS_PER_CORE], F32R, kind="ExternalInput").ap()
    b1 = nc.dram_tensor("b1", [128, 3], F32, kind="ExternalInput").ap()
    w2t0 = nc.dram_tensor("w2t0", [HD, H], F32R, kind="ExternalInput").ap()
    w2t1 = nc.dram_tensor("w2t1", [HD, H], F32R, kind="ExternalInput").ap()
    eye2 = nc.dram_tensor("eye2", [128, HD], F32R, kind="ExternalInput").ap()
    ones2 = nc.dram_tensor("ones2", [128, HD], F32R, kind="ExternalInput").ap()
    out = nc.dram_tensor("out", [BS, H], F32, kind="ExternalOutput").ap()

    NK = H // 128          # 8 contraction chunks for qkv
    NN = BS // 512         # 8 token blocks of 512
    NQB = S // 512         # 4 query blocks per batch
    NKC = S // 128         # 16 key chunks per batch

    with tile.TileContext(nc) as tc, nc.allow_low_precision(reason="fp32r"):
        from contextlib import ExitStack
        with ExitStack() as ctx:
            consts = ctx.enter_context(tc.tile_pool(name="consts", bufs=1))
            mixed = ctx.enter_context(tc.tile_pool(name="mixed", bufs=1))
            ctxp = ctx.enter_context(tc.tile_pool(name="ctxp", bufs=1))
            xtp = ctx.enter_context(tc.tile_pool(name="xtp", bufs=5))
            vsb = ctx.enter_context(tc.tile_pool(name="vsb", bufs=2))
            expp = ctx.enter_context(tc.tile_pool(name="expp", bufs=9))
            sums = ctx.enter_context(tc.tile_pool(name="sums", bufs=2))
            ctxf_p = ctx.enter_context(tc.tile_pool(name="ctxf", bufs=2))
            rbp = ctx.enter_context(tc.tile_pool(name="rbp", bufs=2))
            outs = ctx.enter_context(tc.tile_pool(name="outs", bufs=4))
            ps_sc = ctx.enter_context(tc.tile_pool(name="ps_sc", bufs=2, space="PSUM"))
            ps_ac = ctx.enter_context(tc.tile_pool(name="ps_ac", bufs=2, space="PSUM"))
            ps_ms = ctx.enter_context(tc.tile_pool(name="ps_ms", bufs=2, space="PSUM"))

            # ---- constants ----
            w1big = consts.tile([128, NK, ROWS_PER_CORE], F32R, name="w1big")
            w1r = w1t.rearrange("(k p) r -> p k r", p=128)
            nc.sync.dma_start(w1big[:, 0:1, :], w1r[:, 0:1, :])
            nc.sync.dma_start(w1big[:, 1:NK // 2, :], w1r[:, 1:NK // 2, :])
            nc.sync.dma_start(w1big[:, NK // 2:NK, :], w1r[:, NK // 2:NK, :])
            b1sb = consts.tile([128, 3], F32, name="b1")
            nc.sync.dma_start(b1sb[:], b1)
            warm = consts.tile([1, 1], F32, name="warm")
            nc.scalar.activation(warm[0:1, 0:1], b1sb[0:1, 0:1], Act.Exp)
            eye2sb = consts.tile([128, HD], F32R, name="eye2")
            nc.sync.dma_start(eye2sb[:], eye2)
            ones2sb = consts.tile([128, HD], F32R, name="ones2")
            nc.sync.dma_start(ones2sb[:], ones2)
            w2sb = consts.tile([128, H], F32R, name="w2pack")
            nc.sync.dma_start(w2sb[0:HD, :], w2t0)
            nc.sync.dma_start(w2sb[HD:128, :], w2t1)

            # ---- phase A building blocks ----
            qt = mixed.tile([128, BS], F32R, name="qt")
            kt = mixed.tile([128, BS], F32R, name="kt")
            vt = mixed.tile([128, BS], F32R, name="vt")
            mix_dst = [qt, kt, vt]
            KG = 4  # k-chunks per xt DMA

            def emit_qkv_nblock(n, fine=False):
                """mixedT[:, n*512:(n+1)*512] = W1 @ XT block (+bias).
                m-outer / k-inner: one PSUM slot at a time, PE K-contiguous.
                fine=True splits the loads per k-chunk so the first matmul
                starts as soon as 256KB has landed (kernel warmup)."""
                xts = []
                for kg in range(NK // KG):
                    xt_t = xtp.tile([128, KG, 512], F32R, name="xt")
                    if fine:
                        for c in range(KG):
                            k = kg * KG + c
                            nc.sync.dma_start(
                                xt_t[:, c, :],
                                xt[k * 128:(k + 1) * 128,
                                   n * 512:(n + 1) * 512])
                    else:
                        nc.sync.dma_start(
                            xt_t[:],
                            xt[kg * KG * 128:(kg + 1) * KG * 128,
                               n * 512:(n + 1) * 512].rearrange(
                                   "(c p) f -> p c f", p=128))
                    xts.append(xt_t)
                for m in range(3):
                    ps = ps_ac.tile([128, 512], F32, name=f"qkv{m}", tag="acc")
                    for k in range(NK):
                        nc.tensor.matmul(
                            ps[:],
                            w1big[:, k, m * 128:(m + 1) * 128],
                            xts[k // KG][:, k % KG, :],
                            start=(k == 0), stop=(k == NK - 1))
                    nc.scalar.activation(
                        mix_dst[m][:, n * 512:(n + 1) * 512], ps[:],
                        Act.Identity, bias=b1sb[:, m:m + 1])

            def emit_vprep(b):
                vbig = {}
                for j in range(2):
                    vb = vsb.tile([128, NKC * (HD + 1)], F32R, name=f"vbig{j}")
                    ones_view = vb[:].rearrange(
                        "p (c w) -> p c w", w=HD + 1)[:, :, HD:HD + 1]
                    nc.vector.tensor_copy(ones_view, ones2sb[:, 0:NKC])
                    for kc in range(NKC):
                        pt = ps_ms.tile([128, HD], F32R, name="vtr", tag="misc")
                        nc.tensor.transpose(
                            pt[:],
                            vt[64 * j:64 * j + 64,
                               b * S + kc * 128:b * S + (kc + 1) * 128],
                            eye2sb[64 * j:64 * j + 64, :])
                        nc.vector.tensor_copy(
                            vb[:, kc * (HD + 1):kc * (HD + 1) + HD], pt[:])
                    vbig[j] = vb
                return vbig

            def emit_attention_kc(b, qb, vbig):
                ctxps = {j: ps_ac.tile([HD + 1, 512], F32, name=f"ctxps{j}",
                                       tag="acc")
                         for j in range(2)}
                for kc in range(NKC):
                    sp2 = ps_sc.tile([128, 1024], F32, name="scores")
                    for j in range(2):
                        nc.tensor.matmul(
                            sp2[:, j * 512:(j + 1) * 512],
                            kt[64 * j:64 * j + 64,
                               b * S + kc * 128:b * S + (kc + 1) * 128],
                            qt[64 * j:64 * j + 64,
                               b * S + qb * 512:b * S + (qb + 1) * 512],
                            start=True, stop=True)
                    et2 = expp.tile([128, 1024], F32R, name="exp")
                    nc.scalar.activation(et2[:], sp2[:], Act.Exp, scale=0.125)
                    for j in range(2):
                        nc.tensor.matmul(
                            ctxps[j][:],
                            vbig[j][:, kc * (HD + 1):(kc + 1) * (HD + 1)],
                            et2[:, j * 512:(j + 1) * 512],
                            start=(kc == 0), stop=(kc == NKC - 1))
                return ctxps

            def emit_norm(b, qb, ctxps, cts):
                for j in range(2):
                    # free the accumulator bank after a single copy; the rest
                    # of the normalization runs from SBUF off the critical path
                    ctxf = ctxf_p.tile([HD + 1, 512], F32, name="ctxf")
                    nc.vector.tensor_copy(ctxf[:], ctxps[j][:])
                    ss = sums.tile([1, 512], F32, name="sums")
                    nc.vector.tensor_copy(ss[0:1, :], ctxf[HD:HD + 1, :])
                    rbb = rbp.tile([HD, 512], F32, name="rbb")
                    nc.gpsimd.partition_broadcast(rbb[:], ss[0:1, :])
                    rb = rbp.tile([HD, 512], F32, name="rb")
                    nc.vector.reciprocal_approx_fast(rb[:], rbb[:])
                    nc.vector.tensor_mul(
                        cts[64 * j:64 * (j + 1), qb * 512:(qb + 1) * 512],
                        ctxf[0:HD, :], rb[:])

            def emit_dense_qb(b, qb, cts):
                """Dense partial for the 512-token block qb (4 t-chunks)."""
                for t4 in range(4):
                    t = qb * 4 + t4
                    ob = outs.tile([128, H], F32, name="ostage")
                    for nb in range(2):
                        dp = ps_ms.tile([128, 512], F32, name="dense",
                                        tag="misc")
                        nc.tensor.matmul(
                            dp[:], cts[:, t * 128:(t + 1) * 128],
                            w2sb[:, nb * 512:(nb + 1) * 512],
                            start=True, stop=True)
                        nc.vector.tensor_copy(
                            ob[:, nb * 512:(nb + 1) * 512], dp[:])
                    row0 = b * S + t * 128
                    nc.sync.dma_start(out[row0:row0 + 128, :], ob[:])

            # ---- emission schedule ----
            # Serial qkv projection (DMA-paced; PE slack absorbs the v
            # transposes), then per-batch attention with dense interleaved
            # per query block so the epilogue never piles up at the end.
            for n in range(NN // 2):
                emit_qkv_nblock(n, fine=(n == 0))
            vbigs = {0: emit_vprep(0)}
            cts = {0: ctxp.tile([128, S], F32R, name="ctx_0")}
            pend = (0, 0, emit_attention_kc(0, 0, vbigs[0]))
            for n in range(NN // 2, NN):
                emit_qkv_nblock(n)
            cts[1] = ctxp.tile([128, S], F32R, name="ctx_1")
            for b, qb in [(0, 1), (0, 2), (0, 3),
                          (1, 0), (1, 1), (1, 2), (1, 3)]:
                if (b, qb) == (0, 2):
                    # batch-1 v transposes ride the attention window's spare
                    # PE/misc capacity instead of extending phase A
                    vbigs[1] = emit_vprep(1)
                cur = (b, qb, emit_attention_kc(b, qb, vbigs[b]))
                pb, pq, pctx = pend
                emit_norm(pb, pq, pctx, cts[pb])
                emit_dense_qb(pb, pq, cts[pb])
                pend = cur
            pb, pq, pctx = pend
            emit_norm(pb, pq, pctx, cts[pb])
            emit_dense_qb(pb, pq, cts[pb])
    nc.compile()
    return nc


def _prepare_inputs(hidden_states, qkv_w, qkv_b, dense_w):
    """Build per-core input maps (all host-side slicing/transposition)."""
    x = np.ascontiguousarray(hidden_states, dtype=np.float32).reshape(BS, H)
    xt = _round_fp32r(np.ascontiguousarray(x.T))
    eye2 = np.concatenate([np.eye(HD, dtype=np.float32)] * 2, axis=0)
    ones2 = np.ones((128, HD), dtype=np.float32)
    in_maps = []
    for c in range(NCORES):
        base = c * ROWS_PER_CORE
        # per-head row groups within this core's 384 rows: h0 {q,k,v}, h1 {q,k,v}
        rows = {}
        for m in range(3):  # 0=q 1=k 2=v
            rows[m] = np.r_[base + m * HD:base + (m + 1) * HD,
                            base + 192 + m * HD:base + 192 + (m + 1) * HD]
        perm = np.concatenate([rows[0], rows[1], rows[2]])
        w1t = _round_fp32r(np.ascontiguousarray(qkv_w[perm, :].T))   # [H, 384]
        b1 = np.ascontiguousarray(
            np.stack([qkv_b[rows[m]] for m in range(3)], axis=1),
            dtype=np.float32)                                        # [128, 3]
        w2t0 = _round_fp32r(np.ascontiguousarray(
            dense_w[:, c * DPC:c * DPC + HD].T))                     # [64, 1024]
        w2t1 = _round_fp32r(np.ascontiguousarray(
            dense_w[:, c * DPC + HD:(c + 1) * DPC].T))
        in_maps.append({
            "xt": xt, "w1t": w1t, "b1": b1,
            "w2t0": w2t0, "w2t1": w2t1,
            "eye2": eye2, "ones2": ones2,
        })
    return in_maps


def _reference_numpy(hidden_states, attention_mask, qkv_w, qkv_b, dense_w, dense_b):
    """Exact fallback for non-all-ones masks (never hit with spec inputs)."""
    x = np.asarray(hidden_states, dtype=np.float64)
    mask = np.asarray(attention_mask, dtype=np.float64)
    mixed = x @ np.asarray(qkv_w, np.float64).T + np.asarray(qkv_b, np.float64)
    mixed = mixed.reshape(B, S, NH, 3 * HD).transpose(0, 2, 1, 3)
    q, k, v = np.split(mixed, 3, axis=-1)
    scores = np.einsum("bhqd,bhkd->bhqk", q, k) / np.sqrt(HD)
    scores = scores * mask - 10000.0 * (1.0 - mask)
    scores -= scores.max(axis=-1, keepdims=True)
    probs = np.exp(scores)
    probs /= probs.sum(axis=-1, keepdims=True)
    cx = np.einsum("bhqk,bhkd->bhqd", probs, v)
    cx = cx.transpose(0, 2, 1, 3).reshape(B, S, H)
    o = cx @ np.asarray(dense_w, np.float64).T + np.asarray(dense_b, np.float64)
    return o.astype(np.float32)


def _run(inputs, trace=False):
    from concourse.bass_utils import run_bass_kernel_spmd
    if "nc" not in _CACHE:
        _CACHE["nc"] = _build_program()
    nc = _CACHE["nc"]
    in_maps = _prepare_inputs(inputs["hidden_states"], inputs["qkv_w"],
                              inputs["qkv_b"], inputs["dense_w"])
    res = run_bass_kernel_spmd(nc, in_maps, core_ids=list(range(NCORES)),
                               trace=trace)
    partials = np.stack([r["out"] for r in res.results], axis=0)
    full = partials.sum(axis=0, dtype=np.float64)
    full += np.asarray(inputs["dense_b"], dtype=np.float64)
    return full.astype(np.float32).reshape(B, S, H), res


def kernel(hidden_states, attention_mask, qkv_w, qkv_b, dense_w, dense_b):
    hidden_states = np.asarray(hidden_states)
    attention_mask = np.asarray(attention_mask)
    qkv_w = np.asarray(qkv_w)
    qkv_b = np.asarray(qkv_b)
    dense_w = np.asarray(dense_w)
    dense_b = np.asarray(dense_b)
    if not np.all(attention_mask == 1.0):
        return _reference_numpy(hidden_states, attention_mask, qkv_w, qkv_b,
                                dense_w, dense_b)
    out, _ = _run({
        "hidden_states": hidden_states, "qkv_w": qkv_w, "qkv_b": qkv_b,
        "dense_w": dense_w, "dense_b": dense_b,
    }, trace=bool(int(os.environ.get("KERNEL_TRACE", "0"))))
    return out



# revision 36
# speedup vs baseline: 1.1489x; 1.1489x over previous
"""Trainium2 Bass kernel for nn_Attention: 16-head attention, B=2, S=2048, H=1024.

Megatron-style tensor parallel over heads: 8 cores x 2 heads. Host sums the 8
partial dense outputs (all-reduce-after-dense recipe) and applies the bias
terms that commute out of the kernel.

Per-core dataflow (all matmul inputs bf16, fp32 PSUM accumulation):
  - q,k computed in [dim, token] layout (moving = x^T blocks, ap=512).
  - v computed directly in [token, dim] layout (stationary = x^T chunk,
    moving = v-weights), so no PE transposes are needed for v.
  - scores^T: PSUM [128 keys, 1024] holds two key-chunks x 512 queries; exp
    runs as one [128,1024] instruction, split between the Act engine (Exp
    activation, scale=1/8) and the DVE (pow with constant base e^{1/8}).
  - ctx accumulated in [token, dim] orientation: stationary = probs chunk,
    moving = v chunk with a ones column appended (65th column accumulates the
    softmax denominator for free).
  - late normalization (DVE reciprocal + per-partition scale), PE transpose of
    the normalized ctx to [dim, token], dense with moving = dense weights.
  - engine split: PE matmuls; Act = exp + q/k psum->sbuf (q-bias fused);
    DVE = exp + normalize + reciprocal; Pool = v/ctxT/dense-out copies + DMA
    queues for the streamed x^T blocks and output tiles.
  - bias handling: k-bias is softmax-invariant (dropped), v-bias and dense
    bias are added on the host, q-bias is fused into the q PSUM->SBUF copy.
"""
import math
import os

import numpy as np
import ml_dtypes

B, S, H, NH = 2, 2048, 1024, 16
HD = H // NH             # 64
BS = B * S               # 4096
NCORES = 8
NKK = H // 128           # 8 contraction chunks
NBLK = BS // 512         # 8 token blocks of 512
NQB = S // 512           # 4 query blocks per batch
NKC = S // 128           # 16 key chunks per batch
NPAIR = NKC // 2         # 8 key-chunk pairs per query block
NTC = BS // 128          # 32 token chunks of 128

_CACHE = {}

EXP_BASE = float(np.exp(0.125))  # e^{1/8}; (e^{1/8})^s == exp(s/8)


def _build_program():
    import concourse.mybir as mybir
    import concourse.tile as tile
    from concourse import bacc

    F32 = mybir.dt.float32
    F32R = mybir.dt.float32r
    BF16 = mybir.dt.bfloat16
    Act = mybir.ActivationFunctionType
    Alu = mybir.AluOpType

    nc = bacc.Bacc("TRN2", target_bir_lowering=False, debug=False,
                   num_devices=NCORES)
    xtb = nc.dram_tensor("xtb", [H, BS], BF16, kind="ExternalInput").ap()
    wq = nc.dram_tensor("wq", [128, NKK, 128], BF16, kind="ExternalInput").ap()
    wk = nc.dram_tensor("wk", [128, NKK, 128], BF16, kind="ExternalInput").ap()
    wv = nc.dram_tensor("wv", [128, NKK, 128], BF16, kind="ExternalInput").ap()
    w2m = nc.dram_tensor("w2m", [128, H], BF16, kind="ExternalInput").ap()
    qbias = nc.dram_tensor("qbias", [128, 1], F32, kind="ExternalInput").ap()
    ident = nc.dram_tensor("ident", [128, 128], F32R, kind="ExternalInput").ap()
    out = nc.dram_tensor("out", [BS, H], F32, kind="ExternalOutput").ap()
    dbg = {}
    if os.environ.get("KDBG"):
        dbg["qsb"] = nc.dram_tensor("dbg_qsb", [128, BS], F32, kind="ExternalOutput").ap()
        dbg["ksb"] = nc.dram_tensor("dbg_ksb", [128, BS], F32, kind="ExternalOutput").ap()
        dbg["vsb"] = nc.dram_tensor("dbg_vsb", [128, NTC * 130], F32, kind="ExternalOutput").ap()
        dbg["pt"] = nc.dram_tensor("dbg_pt", [128, 1024], F32, kind="ExternalOutput").ap()
        dbg["ctxa"] = nc.dram_tensor("dbg_ctxa", [128, NQB * 65], F32, kind="ExternalOutput").ap()
        dbg["rec"] = nc.dram_tensor("dbg_rec", [128, NQB], F32, kind="ExternalOutput").ap()
        dbg["ctxt"] = nc.dram_tensor("dbg_ctxt", [128, 16 * 128], F32, kind="ExternalOutput").ap()

    with tile.TileContext(nc) as tc, nc.allow_low_precision(reason="bf16"):
        from contextlib import ExitStack
        with ExitStack() as ctx:
            consts = ctx.enter_context(tc.tile_pool(name="consts", bufs=1))
            persist = ctx.enter_context(tc.tile_pool(name="persist", bufs=1))
            xtp = ctx.enter_context(tc.tile_pool(name="xtp", bufs=7))
            probs = ctx.enter_context(tc.tile_pool(name="probs", bufs=8))
            recipp = ctx.enter_context(tc.tile_pool(name="recipp", bufs=4))
            ctxnp = ctx.enter_context(tc.tile_pool(name="ctxnp", bufs=8))
            ctxtp = ctx.enter_context(tc.tile_pool(name="ctxtp", bufs=2))
            ostage = ctx.enter_context(tc.tile_pool(name="ostage", bufs=3))
            ps_sc = ctx.enter_context(
                tc.tile_pool(name="ps_sc", bufs=2, space="PSUM"))
            ps_ctx = ctx.enter_context(
                tc.tile_pool(name="ps_ctx", bufs=1, space="PSUM"))
            ps_qd = ctx.enter_context(
                tc.tile_pool(name="ps_qd", bufs=2, space="PSUM"))
            ps_ms = ctx.enter_context(
                tc.tile_pool(name="ps_ms", bufs=1, space="PSUM"))

            # ---- constants ----
            # wq first, then the first x^T block, so PE can start ASAP;
            # everything else queues behind on the SP DGE.
            wqsb = consts.tile([128, NKK, 128], BF16, name="wqsb")
            nc.sync.dma_start(wqsb[:, 0:NKK // 2, :], wq[:, 0:NKK // 2, :])
            xt_t0 = xtp.tile([128, NKK, 512], BF16, name="xt")
            src0 = xtb[:, 0:512].rearrange("(k p) t -> p k t", p=128)
            for kk in range(NKK // 2):
                nc.sync.dma_start(xt_t0[:, kk, :], src0[:, kk, :])
            nc.sync.dma_start(wqsb[:, NKK // 2:NKK, :], wq[:, NKK // 2:NKK, :])
            for kk in range(NKK // 2, NKK):
                nc.sync.dma_start(xt_t0[:, kk, :], src0[:, kk, :])
            qbsb = consts.tile([128, 1], F32, name="qbsb")
            nc.sync.dma_start(qbsb[:], qbias)
            wksb = consts.tile([128, NKK, 128], BF16, name="wksb")
            nc.sync.dma_start(wksb[:], wk)
            warm = consts.tile([1, 1], F32, name="warm")
            nc.scalar.activation(warm[0:1, 0:1], qbsb[0:1, 0:1], Act.Exp)

            # ---- persistent q/k/v ----
            qsb = persist.tile([128, BS], BF16, name="qsb")
            ksb = persist.tile([128, BS], BF16, name="ksb")
            # v layout: [token-part, chunk, 2*(64+1)]; cols 64 and 129 hold
            # the ones column that accumulates the softmax denominator.
            vsb = persist.tile([128, NTC, 130], BF16, name="vsb")
            nc.gpsimd.memset(vsb[:], 1.0)

            late_consts = {}

            def emit_late_consts():
                wvsb = consts.tile([128, NKK, 128], BF16, name="wvsb")
                nc.sync.dma_start(wvsb[:], wv)
                w2sb = consts.tile([128, H], BF16, name="w2sb")
                nc.sync.dma_start(w2sb[:], w2m)
                idsb = consts.tile([128, 128], F32R, name="idsb")
                nc.sync.dma_start(idsb[:], ident)
                late_consts.update(wvsb=wvsb, w2sb=w2sb, idsb=idsb)

            # ---- phase A: qkv projection for one 512-token block ----
            # Emitted as a list of small closures ("groups", ~0.6us of PE
            # work each) so blocks 4-7 can interleave into batch-0 attention
            # pairs as exp-independent PE filler.
            xts = {}

            def dma_block(n):
                if n == 0:
                    xts[n] = xt_t0
                    return
                xt_t = xtp.tile([128, NKK, 512], BF16, name="xt")
                src = xtb[:, n * 512:(n + 1) * 512].rearrange(
                    "(k p) t -> p k t", p=128)
                nc.sync.dma_start(xt_t[:], src)
                xts[n] = xt_t

            def qkv_groups(n):
                state = {}

                def g_start():
                    xt_t = state["xt"] = xts[n]
                    qps = ps_qd.tile([128, 512], F32, name="qps", tag="qd")
                    state["qps"] = qps
                    for kk in range(NKK // 2):
                        nc.tensor.matmul(qps[:], wqsb[:, kk, :], xt_t[:, kk, :],
                                         start=(kk == 0), stop=False)

                def g_q2():
                    xt_t, qps = state["xt"], state["qps"]
                    for kk in range(NKK // 2, NKK):
                        nc.tensor.matmul(qps[:], wqsb[:, kk, :], xt_t[:, kk, :],
                                         start=False, stop=(kk == NKK - 1))
                    if n < NBLK // 2:
                        nc.scalar.activation(qsb[:, n * 512:(n + 1) * 512],
                                             qps[:], Act.Identity,
                                             bias=qbsb[:, 0:1])
                    else:
                        nc.vector.tensor_scalar_add(
                            qsb[:, n * 512:(n + 1) * 512], qps[:],
                            qbsb[:, 0:1])

                def g_k1():
                    xt_t = state["xt"]
                    kps = ps_qd.tile([128, 512], F32, name="kps", tag="qd")
                    state["kps"] = kps
                    for kk in range(NKK // 2):
                        nc.tensor.matmul(kps[:], wksb[:, kk, :], xt_t[:, kk, :],
                                         start=(kk == 0), stop=False)

                def g_k2():
                    xt_t, kps = state["xt"], state["kps"]
                    for kk in range(NKK // 2, NKK):
                        nc.tensor.matmul(kps[:], wksb[:, kk, :], xt_t[:, kk, :],
                                         start=False, stop=(kk == NKK - 1))
                    if n < NBLK // 2:
                        nc.scalar.activation(ksb[:, n * 512:(n + 1) * 512],
                                             kps[:], Act.Identity)
                    else:
                        nc.vector.tensor_copy(
                            ksb[:, n * 512:(n + 1) * 512], kps[:])

                def g_v(t4):
                    xt_t = state["xt"]
                    if t4 == 0:
                        state["vps"] = ps_qd.tile([128, 4, 128], F32,
                                                  name="vps", tag="qd")
                    vps = state["vps"]
                    for kk in range(NKK):
                        nc.tensor.matmul(
                            vps[:, t4, :],
                            xt_t[:, kk, t4 * 128:(t4 + 1) * 128],
                            late_consts["wvsb"][:, kk, :],
                            start=(kk == 0 and t4 == 0),
                            stop=(kk == NKK - 1 and t4 == 3))
                    if t4 == 3:
                        for u4 in range(4):
                            g = n * 4 + u4
                            nc.vector.tensor_copy(
                                vsb[:, g, :].rearrange("p (j w) -> p j w",
                                                       w=65)[:, :, 0:64],
                                vps[:, u4, :].rearrange("p (j w) -> p j w",
                                                        w=64))

                return [g_start, g_q2, g_k1, g_k2,
                        lambda: g_v(0), lambda: g_v(1),
                        lambda: g_v(2), lambda: g_v(3)]

            def emit_qkv_block(n):
                dma_block(n)
                for g in qkv_groups(n):
                    g()

            def dbg_dump(name, ap_src, cols):
                if not dbg:
                    return
                st = ostage.tile([128, H], F32, name="dbgst")
                nc.vector.tensor_copy(st[:, 0:cols], ap_src)
                nc.sync.dma_start(dbg[name][:, 0:cols], st[:, 0:cols])

            # ---- phase B helpers ----
            def emit_norm(pend):
                """Reciprocal of the denominators + unnormalized ctx to
                SBUF (normalization folds into the dense-output scale)."""
                b, j, qb, ctxa, ctxt = pend
                den = recipp.tile([128, NQB], F32, name="den", tag="den")
                nc.vector.tensor_copy(den[:], ctxa[:, :, 64])
                rec = recipp.tile([128, NQB], F32, name="rec", tag="rec")
                nc.vector.reciprocal_approx_fast(rec[:], den[:])
                if dbg and (b, j, qb) == (0, 0, 0):
                    dbg_dump("rec", rec[:], NQB)
                cns = [rec]
                for qc in range(NQB):
                    cn = ctxnp.tile([128, 64], F32R, name="cn")
                    nc.vector.tensor_scalar_mul(cn[:], ctxa[:, qc, 0:64],
                                                rec[:, qc:qc + 1])
                    cns.append(cn)
                return cns

            def emit_tp(pend, cns):
                """Transpose normalized ctx to [dim, token] (PE + Pool)."""
                b, j, qb, ctxa, ctxt = pend
                tp = ps_ms.tile([128, NQB, 128], F32R, name="tp", tag="ms")
                for qc in range(NQB):
                    nc.tensor.matmul(tp[0:64, qc, :],
                                     cns[1 + qc][:],
                                     late_consts["idsb"][:],
                                     is_transpose=True,
                                     start=(qc == 0), stop=(qc == NQB - 1))
                for qc in range(NQB):
                    t = qb * 4 + qc
                    nc.vector.tensor_copy(
                        ctxt[j * 64:(j + 1) * 64, t, :],
                        tp[0:64, qc, :])

            def emit_dense_qc(pend, qc, rec, last=False):
                """Dense partial for one 128-token chunk of pend (j==1);
                the softmax normalization is applied here as a per-token
                scale on the PSUM->SBUF drain."""
                b, j, qb, ctxa, ctxt = pend
                t = qb * 4 + qc
                od = ostage.tile([128, H], F32, name="od")
                for nb in range(2):
                    dp = ps_qd.tile([128, 512], F32, name="dp", tag="qd")
                    nc.tensor.matmul(
                        dp[:], ctxt[:, t, :],
                        late_consts["w2sb"][:, nb * 512:(nb + 1) * 512],
                        start=True, stop=True)
                    nc.vector.tensor_copy(
                        od[:, nb * 512:(nb + 1) * 512], dp[:])
                row0 = b * S + t * 128
                nc.sync.dma_start(out[row0:row0 + 128, :], od[:])

            dense_jobs = []    # (pend, qc) waiting for a dense slot
            fill_jobs = []     # qkv groups to interleave as PE filler

            def emit_attention_qb(b, j, qb, ctxt, pend, exp_pat):
                """One 512-query block of head j, batch b. Returns new pend."""
                ctxa = ps_ctx.tile([128, NQB, 65], F32, name="ctxa")
                pts = {}
                for pr in range(NPAIR):
                    sp = ps_sc.tile([128, 1024], F32, name="sp")
                    for hf in range(2):
                        kc = pr * 2 + hf
                        nc.tensor.matmul(
                            sp[:, hf * 512:(hf + 1) * 512],
                            ksb[j * 64:(j + 1) * 64,
                                b * S + kc * 128:b * S + (kc + 1) * 128],
                            qsb[j * 64:(j + 1) * 64,
                                b * S + qb * 512:b * S + (qb + 1) * 512],
                            start=True, stop=True)
                    # exp runs on Act only: the DVE datapath has no exp
                    # and GPSIMD cannot read PSUM on TRN2.
                    pt = probs.tile([128, 1024], BF16, name="pt")
                    nc.scalar.activation(pt[:], sp[:], Act.Exp, scale=0.125)
                    pts[pr] = pt
                    if dbg and (b, j, qb, pr) == (0, 0, 0, 0):
                        dbg_dump("pt", pt[:], 1024)
                    if pr == 2 and pend is not None:
                        emit_tp(pend[:5], pend[5])
                    if pr in (4, 6) and dense_jobs:
                        dpend, qc, drec = dense_jobs.pop(0)
                        emit_dense_qc(dpend, qc, drec)
                    if pr >= 2:
                        emit_ctx(b, j, pr - 2, ctxa, pts.pop(pr - 2))
                    if pr % 2 == 0 and fill_jobs:
                        fill_jobs.pop(0)()
                emit_ctx(b, j, NPAIR - 2, ctxa, pts.pop(NPAIR - 2))
                emit_ctx(b, j, NPAIR - 1, ctxa, pts.pop(NPAIR - 1))
                cur = (b, j, qb, ctxa, ctxt)
                if dbg and (b, j, qb) == (0, 0, 0):
                    dbg_dump("ctxa", ctxa[:].rearrange("p a b -> p (a b)"), NQB * 65)
                cns = emit_norm(cur)
                if j == 1:
                    for qc in range(NQB):
                        dense_jobs.append((cur, qc, cns[0]))
                return cur + (cns,)

            def emit_ctx(b, j, pr, ctxa, pt):
                # a start=True matmul zeroes the whole 2KB PSUM bank, so the
                # four qc sub-accumulators chain into ONE group: only the
                # first matmul starts it, only the last stops it.
                for hf in range(2):
                    kc = pr * 2 + hf
                    for qc in range(NQB):
                        nc.tensor.matmul(
                            ctxa[:, qc, :],
                            pt[:, hf * 512 + qc * 128:hf * 512 + (qc + 1) * 128],
                            vsb[:, b * 16 + kc, j * 65:(j + 1) * 65],
                            start=(kc == 0 and qc == 0),
                            stop=(kc == NKC - 1 and qc == NQB - 1))

            # ---- emission schedule ----
            # Blocks 0-3 (batch 0) up front; blocks 4-7 interleave into
            # batch-0 attention as pair-level PE filler via fill_jobs.
            emit_late_consts()
            for n in range(0, NBLK // 2):
                emit_qkv_block(n)
            # issue batch-1 x^T DMAs now: transfers overlap batch-0 attention
            # (SP/DMA run ahead of PE), compute groups become pair fillers
            for n in range(NBLK // 2, NBLK):
                dma_block(n)
                fill_jobs.extend(qkv_groups(n))
            PAT = ["D", "A"] * 4
            pend = None
            ctxts = {0: ctxtp.tile([128, S // 128, 128], BF16, name="ctxt0")}
            # j interleaved at qb granularity so dense work (which becomes
            # ready only after a j==1 block) spreads across every qb slot.
            for qb in range(NQB):
                for j in range(2):
                    pend = emit_attention_qb(0, j, qb, ctxts[0], pend, PAT)
            while fill_jobs:
                fill_jobs.pop(0)()
            if dbg:
                for cpart in range(BS // 512):
                    dbg_dump("qsb", qsb[:, cpart * 512:(cpart + 1) * 512], 512)
                # overwrite-style: dump full via multiple stages
            if dbg:
                st = ostage.tile([128, H], F32, name="dbgq2")
                for cpart in range(BS // 1024):
                    nc.vector.tensor_copy(st[:], qsb[:, cpart * 1024:(cpart + 1) * 1024])
                    nc.sync.dma_start(dbg["qsb"][:, cpart * 1024:(cpart + 1) * 1024], st[:])
                    nc.vector.tensor_copy(st[:], ksb[:, cpart * 1024:(cpart + 1) * 1024])
                    nc.sync.dma_start(dbg["ksb"][:, cpart * 1024:(cpart + 1) * 1024], st[:])
                vflat = vsb[:].rearrange("p a b -> p (a b)")
                for cpart in range(5):
                    w = min(1024, NTC * 130 - cpart * 1024)
                    nc.vector.tensor_copy(st[:, 0:w], vflat[:, cpart * 1024:cpart * 1024 + w])
                    nc.sync.dma_start(dbg["vsb"][:, cpart * 1024:cpart * 1024 + w], st[:, 0:w])
            ctxts[1] = ctxtp.tile([128, S // 128, 128], BF16, name="ctxt1")
            for qb in range(NQB):
                for j in range(2):
                    pend = emit_attention_qb(1, j, qb, ctxts[1], pend, PAT)
            while fill_jobs:
                fill_jobs.pop(0)()
            emit_tp(pend[:5], pend[5])
            while dense_jobs:
                dpend, qc, drec = dense_jobs.pop(0)
                emit_dense_qc(dpend, qc, drec, last=True)
            if dbg:
                st2 = ostage.tile([128, H], F32, name="dbgct")
                ctf = ctxts[0][:].rearrange("p a b -> p (a b)")
                for cpart in range(2):
                    nc.vector.tensor_copy(st2[:], ctf[:, cpart * 1024:(cpart + 1) * 1024])
                    nc.sync.dma_start(dbg["ctxt"][:, cpart * 1024:(cpart + 1) * 1024], st2[:])
    nc.compile()
    return nc


def _prepare_inputs(hidden_states, qkv_w, qkv_b, dense_w):
    """Build per-core input maps (host-side slicing/packing, all bf16)."""
    bf16 = ml_dtypes.bfloat16
    x = np.ascontiguousarray(hidden_states, dtype=np.float32).reshape(BS, H)
    xtb = np.ascontiguousarray(x.T).astype(bf16)
    ident = np.eye(128, dtype=np.float32)
    qkv_w = np.asarray(qkv_w, dtype=np.float32)
    qkv_b = np.asarray(qkv_b, dtype=np.float32)
    dense_w = np.asarray(dense_w, dtype=np.float32)

    in_maps = []
    m = np.arange(128)
    jj, dd = m // 64, m % 64
    for c in range(NCORES):
        h = 2 * c + jj                      # head index per local dim m
        row_q = h * 192 + dd
        row_k = h * 192 + 64 + dd
        row_v = h * 192 + 128 + dd
        # w?[p, kk, m] = qkv_w[row(m), kk*128 + p]
        wq = np.ascontiguousarray(
            qkv_w[row_q, :].T.reshape(NKK, 128, 128).transpose(1, 0, 2)
        ).astype(bf16)
        wk = np.ascontiguousarray(
            qkv_w[row_k, :].T.reshape(NKK, 128, 128).transpose(1, 0, 2)
        ).astype(bf16)
        wv = np.ascontiguousarray(
            qkv_w[row_v, :].T.reshape(NKK, 128, 128).transpose(1, 0, 2)
        ).astype(bf16)
        # w2m[m, o] = dense_w[o, (2c + m//64)*64 + m%64]
        gcol = h * 64 + dd
        w2m = np.ascontiguousarray(dense_w[:, gcol].T).astype(bf16)
        qb = np.ascontiguousarray(qkv_b[row_q].reshape(128, 1),
                                  dtype=np.float32)
        in_maps.append({
            "xtb": xtb, "wq": wq, "wk": wk, "wv": wv, "w2m": w2m,
            "qbias": qb, "ident": ident,
        })
    return in_maps


def _reference_numpy(hidden_states, attention_mask, qkv_w, qkv_b, dense_w,
                     dense_b):
    """Exact fallback for non-all-ones masks (never hit with spec inputs)."""
    x = np.asarray(hidden_states, dtype=np.float64)
    mask = np.asarray(attention_mask, dtype=np.float64)
    mixed = x @ np.asarray(qkv_w, np.float64).T + np.asarray(qkv_b, np.float64)
    mixed = mixed.reshape(B, S, NH, 3 * HD).transpose(0, 2, 1, 3)
    q, k, v = np.split(mixed, 3, axis=-1)
    scores = np.einsum("bhqd,bhkd->bhqk", q, k) / np.sqrt(HD)
    scores = scores * mask - 10000.0 * (1.0 - mask)
    scores -= scores.max(axis=-1, keepdims=True)
    probs = np.exp(scores)
    probs /= probs.sum(axis=-1, keepdims=True)
    cx = np.einsum("bhqk,bhkd->bhqd", probs, v)
    cx = cx.transpose(0, 2, 1, 3).reshape(B, S, H)
    o = cx @ np.asarray(dense_w, np.float64).T + np.asarray(dense_b, np.float64)
    return o.astype(np.float32)


def _run(inputs, trace=False):
    from concourse.bass_utils import run_bass_kernel_spmd
    if "nc" not in _CACHE:
        _CACHE["nc"] = _build_program()
    nc = _CACHE["nc"]
    in_maps = _prepare_inputs(inputs["hidden_states"], inputs["qkv_w"],
                              inputs["qkv_b"], inputs["dense_w"])
    res = run_bass_kernel_spmd(nc, in_maps, core_ids=list(range(NCORES)),
                               trace=trace)
    partials = np.stack([r["out"] for r in res.results], axis=0)
    full = partials.sum(axis=0, dtype=np.float64)
    qkv_b = np.asarray(inputs["qkv_b"], dtype=np.float64)
    dense_w = np.asarray(inputs["dense_w"], dtype=np.float64)
    g = np.arange(H)
    bv = qkv_b[(g // HD) * 192 + 128 + (g % HD)]
    full += bv @ dense_w.T + np.asarray(inputs["dense_b"], dtype=np.float64)
    return full.astype(np.float32).reshape(B, S, H), res


def kernel(hidden_states, attention_mask, qkv_w, qkv_b, dense_w, dense_b):
    hidden_states = np.asarray(hidden_states)
    attention_mask = np.asarray(attention_mask)
    qkv_w = np.asarray(qkv_w)
    qkv_b = np.asarray(qkv_b)
    dense_w = np.asarray(dense_w)
    dense_b = np.asarray(dense_b)
    if not np.all(attention_mask == 1.0):
        return _reference_numpy(hidden_states, attention_mask, qkv_w, qkv_b,
                                dense_w, dense_b)
    out, _ = _run({
        "hidden_states": hidden_states, "qkv_w": qkv_w, "qkv_b": qkv_b,
        "dense_w": dense_w, "dense_b": dense_b,
    }, trace=bool(int(os.environ.get("KERNEL_TRACE", "0"))))
    return out


# revision 39
# speedup vs baseline: 1.1512x; 1.0020x over previous
"""Trainium2 Bass kernel for nn_Attention: 16-head attention, B=2, S=2048, H=1024.

Megatron-style tensor parallel over heads: 8 cores x 2 heads. Host sums the 8
partial dense outputs (all-reduce-after-dense recipe) and applies the bias
terms that commute out of the kernel.

Per-core dataflow (all matmul inputs bf16, fp32 PSUM accumulation):
  - q,k computed in [dim, token] layout (moving = x^T blocks, ap=512).
  - v computed directly in [token, dim] layout (stationary = x^T chunk,
    moving = v-weights), so no PE transposes are needed for v.
  - scores^T: PSUM [128 keys, 1024] holds two key-chunks x 512 queries; exp
    runs as one [128,1024] instruction, split between the Act engine (Exp
    activation, scale=1/8) and the DVE (pow with constant base e^{1/8}).
  - ctx accumulated in [token, dim] orientation: stationary = probs chunk,
    moving = v chunk with a ones column appended (65th column accumulates the
    softmax denominator for free).
  - late normalization (DVE reciprocal + per-partition scale), PE transpose of
    the normalized ctx to [dim, token], dense with moving = dense weights.
  - engine split: PE matmuls; Act = exp + q/k psum->sbuf (q-bias fused);
    DVE = exp + normalize + reciprocal; Pool = v/ctxT/dense-out copies + DMA
    queues for the streamed x^T blocks and output tiles.
  - bias handling: k-bias is softmax-invariant (dropped), v-bias and dense
    bias are added on the host, q-bias is fused into the q PSUM->SBUF copy.
"""
import math
import os

import numpy as np
import ml_dtypes

B, S, H, NH = 2, 2048, 1024, 16
HD = H // NH             # 64
BS = B * S               # 4096
NCORES = 8
NKK = H // 128           # 8 contraction chunks
NBLK = BS // 512         # 8 token blocks of 512
NQB = S // 512           # 4 query blocks per batch
NKC = S // 128           # 16 key chunks per batch
NPAIR = NKC // 2         # 8 key-chunk pairs per query block
NTC = BS // 128          # 32 token chunks of 128

_CACHE = {}

EXP_BASE = float(np.exp(0.125))  # e^{1/8}; (e^{1/8})^s == exp(s/8)


def _build_program():
    import concourse.mybir as mybir
    import concourse.tile as tile
    from concourse import bacc

    F32 = mybir.dt.float32
    F32R = mybir.dt.float32r
    BF16 = mybir.dt.bfloat16
    Act = mybir.ActivationFunctionType
    Alu = mybir.AluOpType

    nc = bacc.Bacc("TRN2", target_bir_lowering=False, debug=False,
                   num_devices=NCORES)
    xtb = nc.dram_tensor("xtb", [H, BS], BF16, kind="ExternalInput").ap()
    wq = nc.dram_tensor("wq", [128, NKK, 128], BF16, kind="ExternalInput").ap()
    wk = nc.dram_tensor("wk", [128, NKK, 128], BF16, kind="ExternalInput").ap()
    wv = nc.dram_tensor("wv", [128, NKK, 128], BF16, kind="ExternalInput").ap()
    w2m = nc.dram_tensor("w2m", [128, H], BF16, kind="ExternalInput").ap()
    qbias = nc.dram_tensor("qbias", [128, 1], F32, kind="ExternalInput").ap()
    ident = nc.dram_tensor("ident", [128, 128], F32R, kind="ExternalInput").ap()
    out = nc.dram_tensor("out", [BS, H], F32, kind="ExternalOutput").ap()
    dbg = {}
    if os.environ.get("KDBG"):
        dbg["qsb"] = nc.dram_tensor("dbg_qsb", [128, BS], F32, kind="ExternalOutput").ap()
        dbg["ksb"] = nc.dram_tensor("dbg_ksb", [128, BS], F32, kind="ExternalOutput").ap()
        dbg["vsb"] = nc.dram_tensor("dbg_vsb", [128, NTC * 130], F32, kind="ExternalOutput").ap()
        dbg["pt"] = nc.dram_tensor("dbg_pt", [128, 1024], F32, kind="ExternalOutput").ap()
        dbg["ctxa"] = nc.dram_tensor("dbg_ctxa", [128, NQB * 65], F32, kind="ExternalOutput").ap()
        dbg["rec"] = nc.dram_tensor("dbg_rec", [128, NQB], F32, kind="ExternalOutput").ap()
        dbg["ctxt"] = nc.dram_tensor("dbg_ctxt", [128, 16 * 128], F32, kind="ExternalOutput").ap()

    with tile.TileContext(nc) as tc, nc.allow_low_precision(reason="bf16"):
        from contextlib import ExitStack
        with ExitStack() as ctx:
            consts = ctx.enter_context(tc.tile_pool(name="consts", bufs=1))
            persist = ctx.enter_context(tc.tile_pool(name="persist", bufs=1))
            xtp = ctx.enter_context(tc.tile_pool(name="xtp", bufs=7))
            probs = ctx.enter_context(tc.tile_pool(name="probs", bufs=8))
            recipp = ctx.enter_context(tc.tile_pool(name="recipp", bufs=4))
            ctxnp = ctx.enter_context(tc.tile_pool(name="ctxnp", bufs=8))
            ctxtp = ctx.enter_context(tc.tile_pool(name="ctxtp", bufs=2))
            ostage = ctx.enter_context(tc.tile_pool(name="ostage", bufs=3))
            ps_sc = ctx.enter_context(
                tc.tile_pool(name="ps_sc", bufs=2, space="PSUM"))
            ps_ctx = ctx.enter_context(
                tc.tile_pool(name="ps_ctx", bufs=1, space="PSUM"))
            ps_qd = ctx.enter_context(
                tc.tile_pool(name="ps_qd", bufs=2, space="PSUM"))
            ps_ms = ctx.enter_context(
                tc.tile_pool(name="ps_ms", bufs=1, space="PSUM"))

            # ---- constants ----
            # wq first, then the first x^T block, so PE can start ASAP;
            # everything else queues behind on the SP DGE.
            wqsb = consts.tile([128, NKK, 128], BF16, name="wqsb")
            nc.sync.dma_start(wqsb[:, 0:NKK // 2, :], wq[:, 0:NKK // 2, :])
            xt_t0 = xtp.tile([128, NKK, 512], BF16, name="xt")
            src0 = xtb[:, 0:512].rearrange("(k p) t -> p k t", p=128)
            for kk in range(NKK // 2):
                nc.sync.dma_start(xt_t0[:, kk, :], src0[:, kk, :])
            nc.sync.dma_start(wqsb[:, NKK // 2:NKK, :], wq[:, NKK // 2:NKK, :])
            qbsb = consts.tile([128, 1], F32, name="qbsb")
            nc.sync.dma_start(qbsb[:], qbias)
            for kk in range(NKK // 2, NKK):
                nc.sync.dma_start(xt_t0[:, kk, :], src0[:, kk, :])
            wksb = consts.tile([128, NKK, 128], BF16, name="wksb")
            nc.sync.dma_start(wksb[:], wk)
            warm = consts.tile([1, 1], F32, name="warm")
            nc.scalar.activation(warm[0:1, 0:1], qbsb[0:1, 0:1], Act.Exp)

            # ---- persistent q/k/v ----
            qsb = persist.tile([128, BS], BF16, name="qsb")
            ksb = persist.tile([128, BS], BF16, name="ksb")
            # v layout: [token-part, chunk, 2*(64+1)]; cols 64 and 129 hold
            # the ones column that accumulates the softmax denominator.
            vsb = persist.tile([128, NTC, 130], BF16, name="vsb")
            nc.gpsimd.memset(vsb[:], 1.0)

            late_consts = {}

            def emit_late_consts():
                wvsb = consts.tile([128, NKK, 128], BF16, name="wvsb")
                nc.sync.dma_start(wvsb[:], wv)
                w2sb = consts.tile([128, H], BF16, name="w2sb")
                nc.sync.dma_start(w2sb[:], w2m)
                idsb = consts.tile([128, 128], F32R, name="idsb")
                nc.sync.dma_start(idsb[:], ident)
                late_consts.update(wvsb=wvsb, w2sb=w2sb, idsb=idsb)

            # ---- phase A: qkv projection for one 512-token block ----
            # Emitted as a list of small closures ("groups", ~0.6us of PE
            # work each) so blocks 4-7 can interleave into batch-0 attention
            # pairs as exp-independent PE filler.
            xts = {}

            def dma_block(n):
                if n == 0:
                    xts[n] = xt_t0
                    return
                xt_t = xtp.tile([128, NKK, 512], BF16, name="xt")
                src = xtb[:, n * 512:(n + 1) * 512].rearrange(
                    "(k p) t -> p k t", p=128)
                nc.sync.dma_start(xt_t[:], src)
                xts[n] = xt_t

            def qkv_groups(n):
                state = {}

                def g_start():
                    xt_t = state["xt"] = xts[n]
                    qps = ps_qd.tile([128, 512], F32, name="qps", tag="qd")
                    state["qps"] = qps
                    for kk in range(NKK // 2):
                        nc.tensor.matmul(qps[:], wqsb[:, kk, :], xt_t[:, kk, :],
                                         start=(kk == 0), stop=False)

                def g_q2():
                    xt_t, qps = state["xt"], state["qps"]
                    for kk in range(NKK // 2, NKK):
                        nc.tensor.matmul(qps[:], wqsb[:, kk, :], xt_t[:, kk, :],
                                         start=False, stop=(kk == NKK - 1))
                    if n < NBLK // 2:
                        nc.scalar.activation(qsb[:, n * 512:(n + 1) * 512],
                                             qps[:], Act.Identity,
                                             bias=qbsb[:, 0:1])
                    else:
                        nc.vector.tensor_scalar_add(
                            qsb[:, n * 512:(n + 1) * 512], qps[:],
                            qbsb[:, 0:1])

                def g_k1():
                    xt_t = state["xt"]
                    kps = ps_qd.tile([128, 512], F32, name="kps", tag="qd")
                    state["kps"] = kps
                    for kk in range(NKK // 2):
                        nc.tensor.matmul(kps[:], wksb[:, kk, :], xt_t[:, kk, :],
                                         start=(kk == 0), stop=False)

                def g_k2():
                    xt_t, kps = state["xt"], state["kps"]
                    for kk in range(NKK // 2, NKK):
                        nc.tensor.matmul(kps[:], wksb[:, kk, :], xt_t[:, kk, :],
                                         start=False, stop=(kk == NKK - 1))
                    if n < NBLK // 2:
                        nc.scalar.activation(ksb[:, n * 512:(n + 1) * 512],
                                             kps[:], Act.Identity)
                    else:
                        nc.vector.tensor_copy(
                            ksb[:, n * 512:(n + 1) * 512], kps[:])

                def g_v(t4):
                    xt_t = state["xt"]
                    if t4 == 0:
                        state["vps"] = ps_qd.tile([128, 4, 128], F32,
                                                  name="vps", tag="qd")
                    vps = state["vps"]
                    for kk in range(NKK):
                        nc.tensor.matmul(
                            vps[:, t4, :],
                            xt_t[:, kk, t4 * 128:(t4 + 1) * 128],
                            late_consts["wvsb"][:, kk, :],
                            start=(kk == 0 and t4 == 0),
                            stop=(kk == NKK - 1 and t4 == 3))
                    if t4 == 3:
                        for u4 in range(4):
                            g = n * 4 + u4
                            nc.vector.tensor_copy(
                                vsb[:, g, :].rearrange("p (j w) -> p j w",
                                                       w=65)[:, :, 0:64],
                                vps[:, u4, :].rearrange("p (j w) -> p j w",
                                                        w=64))

                return [g_start, g_q2, g_k1, g_k2,
                        lambda: g_v(0), lambda: g_v(1),
                        lambda: g_v(2), lambda: g_v(3)]

            def emit_qkv_block(n):
                dma_block(n)
                for g in qkv_groups(n):
                    g()

            def dbg_dump(name, ap_src, cols):
                if not dbg:
                    return
                st = ostage.tile([128, H], F32, name="dbgst")
                nc.vector.tensor_copy(st[:, 0:cols], ap_src)
                nc.sync.dma_start(dbg[name][:, 0:cols], st[:, 0:cols])

            # ---- phase B helpers ----
            def emit_norm(pend):
                """Reciprocal of the denominators + unnormalized ctx to
                SBUF (normalization folds into the dense-output scale)."""
                b, j, qb, ctxa, ctxt = pend
                den = recipp.tile([128, NQB], F32, name="den", tag="den")
                nc.vector.tensor_copy(den[:], ctxa[:, :, 64])
                rec = recipp.tile([128, NQB], F32, name="rec", tag="rec")
                nc.vector.reciprocal_approx_fast(rec[:], den[:])
                if dbg and (b, j, qb) == (0, 0, 0):
                    dbg_dump("rec", rec[:], NQB)
                cns = [rec]
                for qc in range(NQB):
                    cn = ctxnp.tile([128, 64], F32R, name="cn")
                    nc.vector.tensor_scalar_mul(cn[:], ctxa[:, qc, 0:64],
                                                rec[:, qc:qc + 1])
                    cns.append(cn)
                return cns

            def emit_tp(pend, cns):
                """Transpose normalized ctx to [dim, token] (PE + Pool)."""
                b, j, qb, ctxa, ctxt = pend
                tp = ps_ms.tile([128, NQB, 128], F32R, name="tp", tag="ms")
                for qc in range(NQB):
                    nc.tensor.matmul(tp[0:64, qc, :],
                                     cns[1 + qc][:],
                                     late_consts["idsb"][:],
                                     is_transpose=True,
                                     start=(qc == 0), stop=(qc == NQB - 1))
                for qc in range(NQB):
                    t = qb * 4 + qc
                    nc.vector.tensor_copy(
                        ctxt[j * 64:(j + 1) * 64, t, :],
                        tp[0:64, qc, :])

            def emit_dense_qc(pend, qc, rec, last=False):
                """Dense partial for one 128-token chunk of pend (j==1);
                the softmax normalization is applied here as a per-token
                scale on the PSUM->SBUF drain."""
                b, j, qb, ctxa, ctxt = pend
                t = qb * 4 + qc
                od = ostage.tile([128, H], F32, name="od")
                for nb in range(2):
                    dp = ps_qd.tile([128, 512], F32, name="dp", tag="qd")
                    nc.tensor.matmul(
                        dp[:], ctxt[:, t, :],
                        late_consts["w2sb"][:, nb * 512:(nb + 1) * 512],
                        start=True, stop=True)
                    nc.vector.tensor_copy(
                        od[:, nb * 512:(nb + 1) * 512], dp[:])
                row0 = b * S + t * 128
                nc.sync.dma_start(out[row0:row0 + 128, :], od[:])

            dense_jobs = []    # (pend, qc) waiting for a dense slot
            fill_jobs = []     # qkv groups to interleave as PE filler

            def emit_attention_qb(b, j, qb, ctxt, pend, exp_pat):
                """One 512-query block of head j, batch b. Returns new pend."""
                ctxa = ps_ctx.tile([128, NQB, 65], F32, name="ctxa")
                pts = {}
                for pr in range(NPAIR):
                    sp = ps_sc.tile([128, 1024], F32, name="sp")
                    for hf in range(2):
                        kc = pr * 2 + hf
                        nc.tensor.matmul(
                            sp[:, hf * 512:(hf + 1) * 512],
                            ksb[j * 64:(j + 1) * 64,
                                b * S + kc * 128:b * S + (kc + 1) * 128],
                            qsb[j * 64:(j + 1) * 64,
                                b * S + qb * 512:b * S + (qb + 1) * 512],
                            start=True, stop=True)
                    # exp runs on Act only: the DVE datapath has no exp
                    # and GPSIMD cannot read PSUM on TRN2.
                    pt = probs.tile([128, 1024], BF16, name="pt")
                    nc.scalar.activation(pt[:], sp[:], Act.Exp, scale=0.125)
                    pts[pr] = pt
                    if dbg and (b, j, qb, pr) == (0, 0, 0, 0):
                        dbg_dump("pt", pt[:], 1024)
                    if pr == 2 and pend is not None:
                        emit_tp(pend[:5], pend[5])
                    if pr in (4, 6) and dense_jobs:
                        dpend, qc, drec = dense_jobs.pop(0)
                        emit_dense_qc(dpend, qc, drec)
                    if pr >= 2:
                        emit_ctx(b, j, pr - 2, ctxa, pts.pop(pr - 2))
                    for _ in range(fill_rate[0](pr)):
                        if fill_jobs:
                            fill_jobs.pop(0)()
                emit_ctx(b, j, NPAIR - 2, ctxa, pts.pop(NPAIR - 2))
                emit_ctx(b, j, NPAIR - 1, ctxa, pts.pop(NPAIR - 1))
                cur = (b, j, qb, ctxa, ctxt)
                if dbg and (b, j, qb) == (0, 0, 0):
                    dbg_dump("ctxa", ctxa[:].rearrange("p a b -> p (a b)"), NQB * 65)
                cns = emit_norm(cur)
                if j == 1:
                    for qc in range(NQB):
                        dense_jobs.append((cur, qc, cns[0]))
                return cur + (cns,)

            def emit_ctx(b, j, pr, ctxa, pt):
                # a start=True matmul zeroes the whole 2KB PSUM bank, so the
                # four qc sub-accumulators chain into ONE group: only the
                # first matmul starts it, only the last stops it.
                for hf in range(2):
                    kc = pr * 2 + hf
                    for qc in range(NQB):
                        nc.tensor.matmul(
                            ctxa[:, qc, :],
                            pt[:, hf * 512 + qc * 128:hf * 512 + (qc + 1) * 128],
                            vsb[:, b * 16 + kc, j * 65:(j + 1) * 65],
                            start=(kc == 0 and qc == 0),
                            stop=(kc == NKC - 1 and qc == NQB - 1))

            # ---- emission schedule ----
            # Blocks 0-3 (batch 0) up front; blocks 4-7 interleave into
            # batch-0 attention as pair-level PE filler via fill_jobs.
            emit_late_consts()
            for n in range(0, NBLK // 2):
                emit_qkv_block(n)
            # issue batch-1 x^T DMAs now: transfers overlap batch-0 attention
            # (SP/DMA run ahead of PE), compute groups become pair fillers
            for n in range(NBLK // 2, NBLK):
                dma_block(n)
                fill_jobs.extend(qkv_groups(n))
            PAT = ["D", "A"] * 4
            fill_rate = [lambda pr: 0]
            pend = None
            ctxts = {0: ctxtp.tile([128, S // 128, 128], BF16, name="ctxt0")}
            # j interleaved at qb granularity so dense work (which becomes
            # ready only after a j==1 block) spreads across every qb slot.
            fill_rate[0] = lambda pr: 1 if pr in (0, 2, 4) else 0
            for qb in range(NQB):
                for j in range(2):
                    pend = emit_attention_qb(0, j, qb, ctxts[0], pend, PAT)
            if dbg:
                for cpart in range(BS // 512):
                    dbg_dump("qsb", qsb[:, cpart * 512:(cpart + 1) * 512], 512)
                # overwrite-style: dump full via multiple stages
            if dbg:
                st = ostage.tile([128, H], F32, name="dbgq2")
                for cpart in range(BS // 1024):
                    nc.vector.tensor_copy(st[:], qsb[:, cpart * 1024:(cpart + 1) * 1024])
                    nc.sync.dma_start(dbg["qsb"][:, cpart * 1024:(cpart + 1) * 1024], st[:])
                    nc.vector.tensor_copy(st[:], ksb[:, cpart * 1024:(cpart + 1) * 1024])
                    nc.sync.dma_start(dbg["ksb"][:, cpart * 1024:(cpart + 1) * 1024], st[:])
                vflat = vsb[:].rearrange("p a b -> p (a b)")
                for cpart in range(5):
                    w = min(1024, NTC * 130 - cpart * 1024)
                    nc.vector.tensor_copy(st[:, 0:w], vflat[:, cpart * 1024:cpart * 1024 + w])
                    nc.sync.dma_start(dbg["vsb"][:, cpart * 1024:cpart * 1024 + w], st[:, 0:w])
            ctxts[1] = ctxtp.tile([128, S // 128, 128], BF16, name="ctxt1")
            fill_rate[0] = lambda pr: 1
            pend = emit_attention_qb(1, 0, 0, ctxts[1], pend, PAT)
            fill_rate[0] = lambda pr: 0
            while fill_jobs:
                fill_jobs.pop(0)()
            for qb in range(NQB):
                for j in range(2):
                    if (j, qb) == (0, 0):
                        continue
                    pend = emit_attention_qb(1, j, qb, ctxts[1], pend, PAT)
            while fill_jobs:
                fill_jobs.pop(0)()
            emit_tp(pend[:5], pend[5])
            while dense_jobs:
                dpend, qc, drec = dense_jobs.pop(0)
                emit_dense_qc(dpend, qc, drec, last=True)
            if dbg:
                st2 = ostage.tile([128, H], F32, name="dbgct")
                ctf = ctxts[0][:].rearrange("p a b -> p (a b)")
                for cpart in range(2):
                    nc.vector.tensor_copy(st2[:], ctf[:, cpart * 1024:(cpart + 1) * 1024])
                    nc.sync.dma_start(dbg["ctxt"][:, cpart * 1024:(cpart + 1) * 1024], st2[:])
    nc.compile()
    return nc


def _prepare_inputs(hidden_states, qkv_w, qkv_b, dense_w):
    """Build per-core input maps (host-side slicing/packing, all bf16)."""
    bf16 = ml_dtypes.bfloat16
    x = np.ascontiguousarray(hidden_states, dtype=np.float32).reshape(BS, H)
    xtb = np.ascontiguousarray(x.T).astype(bf16)
    ident = np.eye(128, dtype=np.float32)
    qkv_w = np.asarray(qkv_w, dtype=np.float32)
    qkv_b = np.asarray(qkv_b, dtype=np.float32)
    dense_w = np.asarray(dense_w, dtype=np.float32)

    in_maps = []
    m = np.arange(128)
    jj, dd = m // 64, m % 64
    for c in range(NCORES):
        h = 2 * c + jj                      # head index per local dim m
        row_q = h * 192 + dd
        row_k = h * 192 + 64 + dd
        row_v = h * 192 + 128 + dd
        # w?[p, kk, m] = qkv_w[row(m), kk*128 + p]
        wq = np.ascontiguousarray(
            qkv_w[row_q, :].T.reshape(NKK, 128, 128).transpose(1, 0, 2)
        ).astype(bf16)
        wk = np.ascontiguousarray(
            qkv_w[row_k, :].T.reshape(NKK, 128, 128).transpose(1, 0, 2)
        ).astype(bf16)
        wv = np.ascontiguousarray(
            qkv_w[row_v, :].T.reshape(NKK, 128, 128).transpose(1, 0, 2)
        ).astype(bf16)
        # w2m[m, o] = dense_w[o, (2c + m//64)*64 + m%64]
        gcol = h * 64 + dd
        w2m = np.ascontiguousarray(dense_w[:, gcol].T).astype(bf16)
        qb = np.ascontiguousarray(qkv_b[row_q].reshape(128, 1),
                                  dtype=np.float32)
        in_maps.append({
            "xtb": xtb, "wq": wq, "wk": wk, "wv": wv, "w2m": w2m,
            "qbias": qb, "ident": ident,
        })
    return in_maps


def _reference_numpy(hidden_states, attention_mask, qkv_w, qkv_b, dense_w,
                     dense_b):
    """Exact fallback for non-all-ones masks (never hit with spec inputs)."""
    x = np.asarray(hidden_states, dtype=np.float64)
    mask = np.asarray(attention_mask, dtype=np.float64)
    mixed = x @ np.asarray(qkv_w, np.float64).T + np.asarray(qkv_b, np.float64)
    mixed = mixed.reshape(B, S, NH, 3 * HD).transpose(0, 2, 1, 3)
    q, k, v = np.split(mixed, 3, axis=-1)
    scores = np.einsum("bhqd,bhkd->bhqk", q, k) / np.sqrt(HD)
    scores = scores * mask - 10000.0 * (1.0 - mask)
    scores -= scores.max(axis=-1, keepdims=True)
    probs = np.exp(scores)
    probs /= probs.sum(axis=-1, keepdims=True)
    cx = np.einsum("bhqk,bhkd->bhqd", probs, v)
    cx = cx.transpose(0, 2, 1, 3).reshape(B, S, H)
    o = cx @ np.asarray(dense_w, np.float64).T + np.asarray(dense_b, np.float64)
    return o.astype(np.float32)


def _run(inputs, trace=False):
    from concourse.bass_utils import run_bass_kernel_spmd
    if "nc" not in _CACHE:
        _CACHE["nc"] = _build_program()
    nc = _CACHE["nc"]
    in_maps = _prepare_inputs(inputs["hidden_states"], inputs["qkv_w"],
                              inputs["qkv_b"], inputs["dense_w"])
    res = run_bass_kernel_spmd(nc, in_maps, core_ids=list(range(NCORES)),
                               trace=trace)
    partials = np.stack([r["out"] for r in res.results], axis=0)
    full = partials.sum(axis=0, dtype=np.float64)
    qkv_b = np.asarray(inputs["qkv_b"], dtype=np.float64)
    dense_w = np.asarray(inputs["dense_w"], dtype=np.float64)
    g = np.arange(H)
    bv = qkv_b[(g // HD) * 192 + 128 + (g % HD)]
    full += bv @ dense_w.T + np.asarray(inputs["dense_b"], dtype=np.float64)
    return full.astype(np.float32).reshape(B, S, H), res


def kernel(hidden_states, attention_mask, qkv_w, qkv_b, dense_w, dense_b):
    hidden_states = np.asarray(hidden_states)
    attention_mask = np.asarray(attention_mask)
    qkv_w = np.asarray(qkv_w)
    qkv_b = np.asarray(qkv_b)
    dense_w = np.asarray(dense_w)
    dense_b = np.asarray(dense_b)
    if not np.all(attention_mask == 1.0):
        return _reference_numpy(hidden_states, attention_mask, qkv_w, qkv_b,
                                dense_w, dense_b)
    out, _ = _run({
        "hidden_states": hidden_states, "qkv_w": qkv_w, "qkv_b": qkv_b,
        "dense_w": dense_w, "dense_b": dense_b,
    }, trace=bool(int(os.environ.get("KERNEL_TRACE", "0"))))
    return out


# revision 47
# speedup vs baseline: 1.1687x; 1.0152x over previous
"""Trainium2 Bass kernel for nn_Attention: 16-head attention, B=2, S=2048, H=1024.

Megatron-style tensor parallel over heads: 8 cores x 2 heads. Host sums the 8
partial dense outputs (all-reduce-after-dense recipe) and applies the bias
terms that commute out of the kernel.

Per-core dataflow (all matmul inputs bf16, fp32 PSUM accumulation):
  - q,k computed in [dim, token] layout (moving = x^T blocks, ap=512).
  - v computed directly in [token, dim] layout (stationary = x^T chunk,
    moving = v-weights), so no PE transposes are needed for v.
  - scores^T: PSUM [128 keys, 1024] holds two key-chunks x 512 queries; exp
    runs as one [128,1024] instruction, split between the Act engine (Exp
    activation, scale=1/8) and the DVE (pow with constant base e^{1/8}).
  - ctx accumulated in [token, dim] orientation: stationary = probs chunk,
    moving = v chunk with a ones column appended (65th column accumulates the
    softmax denominator for free).
  - late normalization (DVE reciprocal + per-partition scale), PE transpose of
    the normalized ctx to [dim, token], dense with moving = dense weights.
  - engine split: PE matmuls; Act = exp + q/k psum->sbuf (q-bias fused);
    DVE = exp + normalize + reciprocal; Pool = v/ctxT/dense-out copies + DMA
    queues for the streamed x^T blocks and output tiles.
  - bias handling: k-bias is softmax-invariant (dropped), v-bias and dense
    bias are added on the host, q-bias is fused into the q PSUM->SBUF copy.
"""
import math
import os

import numpy as np
import ml_dtypes

B, S, H, NH = 2, 2048, 1024, 16
HD = H // NH             # 64
BS = B * S               # 4096
NCORES = 8
NKK = H // 128           # 8 contraction chunks
NBLK = BS // 512         # 8 token blocks of 512
NQB = S // 512           # 4 query blocks per batch
NKC = S // 128           # 16 key chunks per batch
NPAIR = NKC // 2         # 8 key-chunk pairs per query block
NTC = BS // 128          # 32 token chunks of 128

_CACHE = {}

EXP_BASE = float(np.exp(0.125))  # e^{1/8}; (e^{1/8})^s == exp(s/8)


def _build_program():
    import concourse.mybir as mybir
    import concourse.tile as tile
    from concourse import bacc

    F32 = mybir.dt.float32
    F32R = mybir.dt.float32r
    BF16 = mybir.dt.bfloat16
    Act = mybir.ActivationFunctionType
    Alu = mybir.AluOpType

    nc = bacc.Bacc("TRN2", target_bir_lowering=False, debug=False,
                   num_devices=NCORES)
    xtb = nc.dram_tensor("xtb", [H, BS], BF16, kind="ExternalInput").ap()
    wq = nc.dram_tensor("wq", [128, NKK, 128], BF16, kind="ExternalInput").ap()
    wk = nc.dram_tensor("wk", [128, NKK, 128], BF16, kind="ExternalInput").ap()
    wv = nc.dram_tensor("wv", [128, NKK, 128], BF16, kind="ExternalInput").ap()
    w2m = nc.dram_tensor("w2m", [128, H], BF16, kind="ExternalInput").ap()
    qbias = nc.dram_tensor("qbias", [128, 1], F32, kind="ExternalInput").ap()
    ident = nc.dram_tensor("ident", [128, 128], F32R, kind="ExternalInput").ap()
    out = nc.dram_tensor("out", [BS, H], F32, kind="ExternalOutput").ap()
    dbg = {}
    if os.environ.get("KDBG"):
        dbg["qsb"] = nc.dram_tensor("dbg_qsb", [128, BS], F32, kind="ExternalOutput").ap()
        dbg["ksb"] = nc.dram_tensor("dbg_ksb", [128, BS], F32, kind="ExternalOutput").ap()
        dbg["vsb"] = nc.dram_tensor("dbg_vsb", [128, NTC * 130], F32, kind="ExternalOutput").ap()
        dbg["pt"] = nc.dram_tensor("dbg_pt", [128, 1024], F32, kind="ExternalOutput").ap()
        dbg["ctxa"] = nc.dram_tensor("dbg_ctxa", [128, NQB * 65], F32, kind="ExternalOutput").ap()
        dbg["rec"] = nc.dram_tensor("dbg_rec", [128, NQB], F32, kind="ExternalOutput").ap()
        dbg["ctxt"] = nc.dram_tensor("dbg_ctxt", [128, 16 * 128], F32, kind="ExternalOutput").ap()

    with tile.TileContext(nc) as tc, nc.allow_low_precision(reason="bf16"):
        from contextlib import ExitStack
        with ExitStack() as ctx:
            consts = ctx.enter_context(tc.tile_pool(name="consts", bufs=1))
            persist = ctx.enter_context(tc.tile_pool(name="persist", bufs=1))
            xtp = ctx.enter_context(tc.tile_pool(name="xtp", bufs=8))
            probs = ctx.enter_context(tc.tile_pool(name="probs", bufs=10))
            recipp = ctx.enter_context(tc.tile_pool(name="recipp", bufs=4))
            ctxnp = ctx.enter_context(tc.tile_pool(name="ctxnp", bufs=12))
            ctxtp = ctx.enter_context(tc.tile_pool(name="ctxtp", bufs=2))
            ostage = ctx.enter_context(tc.tile_pool(name="ostage", bufs=4))
            ps_sc = ctx.enter_context(
                tc.tile_pool(name="ps_sc", bufs=2, space="PSUM"))
            ps_ctx = ctx.enter_context(
                tc.tile_pool(name="ps_ctx", bufs=1, space="PSUM"))
            ps_qd = ctx.enter_context(
                tc.tile_pool(name="ps_qd", bufs=2, space="PSUM"))
            ps_ms = ctx.enter_context(
                tc.tile_pool(name="ps_ms", bufs=1, space="PSUM"))

            # ---- constants ----
            # wq first, then the first x^T block, so PE can start ASAP;
            # everything else queues behind on the SP DGE.
            wqsb = consts.tile([128, NKK, 128], BF16, name="wqsb")
            nc.sync.dma_start(wqsb[:, 0:NKK // 2, :], wq[:, 0:NKK // 2, :])
            xt_t0 = xtp.tile([128, NKK, 512], BF16, name="xt")
            src0 = xtb[:, 0:512].rearrange("(k p) t -> p k t", p=128)
            for kk in range(NKK // 2):
                nc.sync.dma_start(xt_t0[:, kk, :], src0[:, kk, :])
            nc.sync.dma_start(wqsb[:, NKK // 2:NKK, :], wq[:, NKK // 2:NKK, :])
            qbsb = consts.tile([128, 1], F32, name="qbsb")
            nc.sync.dma_start(qbsb[:], qbias)
            for kk in range(NKK // 2, NKK):
                nc.sync.dma_start(xt_t0[:, kk, :], src0[:, kk, :])
            wksb = consts.tile([128, NKK, 128], BF16, name="wksb")
            nc.sync.dma_start(wksb[:], wk)
            warm = consts.tile([1, 1], F32, name="warm")
            nc.scalar.activation(warm[0:1, 0:1], qbsb[0:1, 0:1], Act.Exp)

            # ---- persistent q/k/v ----
            qsb = persist.tile([128, BS], BF16, name="qsb")
            ksb = persist.tile([128, BS], BF16, name="ksb")
            # v layout: [token-part, chunk, 2*(64+1)]; cols 64 and 129 hold
            # the ones column that accumulates the softmax denominator.
            vsb = persist.tile([128, NTC, 130], BF16, name="vsb")
            nc.gpsimd.memset(vsb[:], 1.0)

            late_consts = {}

            def emit_late_consts():
                wvsb = consts.tile([128, NKK, 128], BF16, name="wvsb")
                nc.sync.dma_start(wvsb[:], wv)
                w2sb = consts.tile([128, H], BF16, name="w2sb")
                nc.sync.dma_start(w2sb[:], w2m)
                idsb = consts.tile([128, 128], F32R, name="idsb")
                nc.sync.dma_start(idsb[:], ident)
                late_consts.update(wvsb=wvsb, w2sb=w2sb, idsb=idsb)

            # ---- phase A: qkv projection for one 512-token block ----
            # Emitted as a list of small closures ("groups", ~0.6us of PE
            # work each) so blocks 4-7 can interleave into batch-0 attention
            # pairs as exp-independent PE filler.
            xts = {}

            def dma_block(n):
                if n == 0:
                    xts[n] = xt_t0
                    return
                xt_t = xtp.tile([128, NKK, 512], BF16, name="xt")
                src = xtb[:, n * 512:(n + 1) * 512].rearrange(
                    "(k p) t -> p k t", p=128)
                nc.sync.dma_start(xt_t[:], src)
                xts[n] = xt_t

            def qkv_groups(n):
                state = {}

                def g_start():
                    xt_t = state["xt"] = xts[n]
                    qps = ps_qd.tile([128, 512], F32, name="qps", tag="qd")
                    state["qps"] = qps
                    for kk in range(NKK // 2):
                        nc.tensor.matmul(qps[:], wqsb[:, kk, :], xt_t[:, kk, :],
                                         start=(kk == 0), stop=False)

                def g_q2():
                    xt_t, qps = state["xt"], state["qps"]
                    for kk in range(NKK // 2, NKK):
                        nc.tensor.matmul(qps[:], wqsb[:, kk, :], xt_t[:, kk, :],
                                         start=False, stop=(kk == NKK - 1))
                    if n < NBLK // 2:
                        nc.scalar.activation(qsb[:, n * 512:(n + 1) * 512],
                                             qps[:], Act.Identity,
                                             bias=qbsb[:, 0:1])
                    else:
                        nc.vector.tensor_scalar_add(
                            qsb[:, n * 512:(n + 1) * 512], qps[:],
                            qbsb[:, 0:1])

                def g_k1():
                    xt_t = state["xt"]
                    kps = ps_qd.tile([128, 512], F32, name="kps", tag="qd")
                    state["kps"] = kps
                    for kk in range(NKK // 2):
                        nc.tensor.matmul(kps[:], wksb[:, kk, :], xt_t[:, kk, :],
                                         start=(kk == 0), stop=False)

                def g_k2():
                    xt_t, kps = state["xt"], state["kps"]
                    for kk in range(NKK // 2, NKK):
                        nc.tensor.matmul(kps[:], wksb[:, kk, :], xt_t[:, kk, :],
                                         start=False, stop=(kk == NKK - 1))
                    if n < NBLK // 2:
                        nc.scalar.activation(ksb[:, n * 512:(n + 1) * 512],
                                             kps[:], Act.Identity)
                    else:
                        nc.vector.tensor_copy(
                            ksb[:, n * 512:(n + 1) * 512], kps[:])

                def g_v(t4):
                    xt_t = state["xt"]
                    if t4 == 0:
                        state["vps"] = ps_qd.tile([128, 4, 128], F32,
                                                  name="vps", tag="qd")
                    vps = state["vps"]
                    for kk in range(NKK):
                        nc.tensor.matmul(
                            vps[:, t4, :],
                            xt_t[:, kk, t4 * 128:(t4 + 1) * 128],
                            late_consts["wvsb"][:, kk, :],
                            start=(kk == 0 and t4 == 0),
                            stop=(kk == NKK - 1 and t4 == 3))
                    if t4 == 3:
                        for u4 in range(4):
                            g = n * 4 + u4
                            nc.vector.tensor_copy(
                                vsb[:, g, :].rearrange("p (j w) -> p j w",
                                                       w=65)[:, :, 0:64],
                                vps[:, u4, :].rearrange("p (j w) -> p j w",
                                                        w=64))

                return [g_start, g_q2, g_k1, g_k2,
                        lambda: g_v(0), lambda: g_v(1),
                        lambda: g_v(2), lambda: g_v(3)]

            def emit_qkv_block(n):
                dma_block(n)
                for g in qkv_groups(n):
                    g()

            def dbg_dump(name, ap_src, cols):
                if not dbg:
                    return
                st = ostage.tile([128, H], F32, name="dbgst")
                nc.vector.tensor_copy(st[:, 0:cols], ap_src)
                nc.sync.dma_start(dbg[name][:, 0:cols], st[:, 0:cols])

            # ---- phase B helpers ----
            def emit_norm(pend):
                """Reciprocal of the denominators + unnormalized ctx to
                SBUF (normalization folds into the dense-output scale)."""
                b, j, qb, ctxa, ctxt = pend
                den = recipp.tile([128, NQB], F32, name="den", tag="den")
                nc.vector.tensor_copy(den[:], ctxa[:, :, 64])
                rec = recipp.tile([128, NQB], F32, name="rec", tag="rec")
                nc.vector.reciprocal_approx_fast(rec[:], den[:])
                if dbg and (b, j, qb) == (0, 0, 0):
                    dbg_dump("rec", rec[:], NQB)
                cns = [rec]
                for qc in range(NQB):
                    cn = ctxnp.tile([128, 64], F32R, name="cn")
                    nc.vector.tensor_scalar_mul(cn[:], ctxa[:, qc, 0:64],
                                                rec[:, qc:qc + 1])
                    cns.append(cn)
                return cns

            def emit_tp(pend, cns):
                """Transpose normalized ctx to [dim, token] (PE + Pool)."""
                b, j, qb, ctxa, ctxt = pend
                tp = ps_ms.tile([128, NQB, 128], F32R, name="tp", tag="ms")
                for qc in range(NQB):
                    nc.tensor.matmul(tp[0:64, qc, :],
                                     cns[1 + qc][:],
                                     late_consts["idsb"][:],
                                     is_transpose=True,
                                     start=(qc == 0), stop=(qc == NQB - 1))
                for qc in range(NQB):
                    t = qb * 4 + qc
                    nc.vector.tensor_copy(
                        ctxt[j * 64:(j + 1) * 64, t, :],
                        tp[0:64, qc, :])

            def emit_dense_qc(pend, qc, rec, last=False):
                """Dense partial for one 128-token chunk of pend (j==1);
                the softmax normalization is applied here as a per-token
                scale on the PSUM->SBUF drain."""
                b, j, qb, ctxa, ctxt = pend
                t = qb * 4 + qc
                od = ostage.tile([128, H], F32, name="od")
                for nb in range(2):
                    dp = ps_qd.tile([128, 512], F32, name="dp", tag="qd")
                    nc.tensor.matmul(
                        dp[:], ctxt[:, t, :],
                        late_consts["w2sb"][:, nb * 512:(nb + 1) * 512],
                        start=True, stop=True)
                    nc.vector.tensor_copy(
                        od[:, nb * 512:(nb + 1) * 512], dp[:])
                row0 = b * S + t * 128
                nc.sync.dma_start(out[row0:row0 + 128, :], od[:])

            dense_jobs = []    # (pend, qc) waiting for a dense slot
            fill_jobs = []     # qkv groups to interleave as PE filler

            def emit_attention_qb(b, j, qb, ctxt, pend, exp_pat):
                """One 512-query block of head j, batch b. Returns new pend."""
                ctxa = ps_ctx.tile([128, NQB, 65], F32, name="ctxa")
                pts = {}
                for pr in range(NPAIR):
                    for _ in range(fill_rate[0](pr)):
                        if fill_jobs:
                            fill_jobs.pop(0)()
                    sp = ps_sc.tile([128, 1024], F32, name="sp")
                    for hf in range(2):
                        kc = pr * 2 + hf
                        nc.tensor.matmul(
                            sp[:, hf * 512:(hf + 1) * 512],
                            ksb[j * 64:(j + 1) * 64,
                                b * S + kc * 128:b * S + (kc + 1) * 128],
                            qsb[j * 64:(j + 1) * 64,
                                b * S + qb * 512:b * S + (qb + 1) * 512],
                            start=True, stop=True)
                    # exp runs on Act only: the DVE datapath has no exp
                    # and GPSIMD cannot read PSUM on TRN2.
                    pt = probs.tile([128, 1024], BF16, name="pt")
                    nc.scalar.activation(pt[:], sp[:], Act.Exp, scale=0.125)
                    pts[pr] = pt
                    if dbg and (b, j, qb, pr) == (0, 0, 0, 0):
                        dbg_dump("pt", pt[:], 1024)
                    if pr == 2 and pend is not None:
                        emit_tp(pend[:5], pend[5])
                    if pr in (4, 6) and dense_jobs:
                        dpend, qc, drec = dense_jobs.pop(0)
                        emit_dense_qc(dpend, qc, drec)
                    if pr >= 2:
                        emit_ctx(b, j, pr - 2, ctxa, pts.pop(pr - 2))
                emit_ctx(b, j, NPAIR - 2, ctxa, pts.pop(NPAIR - 2))
                emit_ctx(b, j, NPAIR - 1, ctxa, pts.pop(NPAIR - 1))
                cur = (b, j, qb, ctxa, ctxt)
                if dbg and (b, j, qb) == (0, 0, 0):
                    dbg_dump("ctxa", ctxa[:].rearrange("p a b -> p (a b)"), NQB * 65)
                cns = emit_norm(cur)
                if j == 1:
                    for qc in range(NQB):
                        dense_jobs.append((cur, qc, cns[0]))
                return cur + (cns,)

            def emit_ctx(b, j, pr, ctxa, pt):
                # a start=True matmul zeroes the whole 2KB PSUM bank, so the
                # four qc sub-accumulators chain into ONE group: only the
                # first matmul starts it, only the last stops it.
                for hf in range(2):
                    kc = pr * 2 + hf
                    for qc in range(NQB):
                        nc.tensor.matmul(
                            ctxa[:, qc, :],
                            pt[:, hf * 512 + qc * 128:hf * 512 + (qc + 1) * 128],
                            vsb[:, b * 16 + kc, j * 65:(j + 1) * 65],
                            start=(kc == 0 and qc == 0),
                            stop=(kc == NKC - 1 and qc == NQB - 1))

            # ---- emission schedule ----
            # Blocks 0-3 (batch 0) up front; blocks 4-7 interleave into
            # batch-0 attention as pair-level PE filler via fill_jobs.
            emit_late_consts()
            emit_qkv_block(0)
            # blocks 1-7: DMAs issued now (transfers run ahead of PE); the
            # compute groups interleave into attention as pair-level filler.
            # Attention on (batch 0, qb 0) only needs k/v blocks progressively
            # (scores for key-chunk kc reads k-block kc//4), so it starts
            # right after block 0.
            for n in range(1, NBLK):
                dma_block(n)
                fill_jobs.extend(qkv_groups(n))
            PAT = ["D", "A"] * 4
            fill_rate = [lambda pr: 0]
            pend = None
            ctxts = {0: ctxtp.tile([128, S // 128, 128], BF16, name="ctxt0")}
            # j interleaved at qb granularity so dense work (which becomes
            # ready only after a j==1 block) spreads across every qb slot.
            QB0_RATE = [6, 6, 4, 4, 2, 2, 0, 0]
            for qb in range(NQB):
                for j in range(2):
                    if (j, qb) == (0, 0):
                        fill_rate[0] = lambda pr: QB0_RATE[pr]
                    else:
                        fill_rate[0] = lambda pr: 1 if pr in (0, 2, 4, 6) else 0
                    pend = emit_attention_qb(0, j, qb, ctxts[0], pend, PAT)
            if dbg:
                for cpart in range(BS // 512):
                    dbg_dump("qsb", qsb[:, cpart * 512:(cpart + 1) * 512], 512)
                # overwrite-style: dump full via multiple stages
            if dbg:
                st = ostage.tile([128, H], F32, name="dbgq2")
                for cpart in range(BS // 1024):
                    nc.vector.tensor_copy(st[:], qsb[:, cpart * 1024:(cpart + 1) * 1024])
                    nc.sync.dma_start(dbg["qsb"][:, cpart * 1024:(cpart + 1) * 1024], st[:])
                    nc.vector.tensor_copy(st[:], ksb[:, cpart * 1024:(cpart + 1) * 1024])
                    nc.sync.dma_start(dbg["ksb"][:, cpart * 1024:(cpart + 1) * 1024], st[:])
                vflat = vsb[:].rearrange("p a b -> p (a b)")
                for cpart in range(5):
                    w = min(1024, NTC * 130 - cpart * 1024)
                    nc.vector.tensor_copy(st[:, 0:w], vflat[:, cpart * 1024:cpart * 1024 + w])
                    nc.sync.dma_start(dbg["vsb"][:, cpart * 1024:cpart * 1024 + w], st[:, 0:w])
            ctxts[1] = ctxtp.tile([128, S // 128, 128], BF16, name="ctxt1")
            fill_rate[0] = lambda pr: 1
            pend = emit_attention_qb(1, 0, 0, ctxts[1], pend, PAT)
            fill_rate[0] = lambda pr: 0
            while fill_jobs:
                fill_jobs.pop(0)()
            for qb in range(NQB):
                for j in range(2):
                    if (j, qb) == (0, 0):
                        continue
                    pend = emit_attention_qb(1, j, qb, ctxts[1], pend, PAT)
            while fill_jobs:
                fill_jobs.pop(0)()
            emit_tp(pend[:5], pend[5])
            while dense_jobs:
                dpend, qc, drec = dense_jobs.pop(0)
                emit_dense_qc(dpend, qc, drec, last=True)
            if dbg:
                st2 = ostage.tile([128, H], F32, name="dbgct")
                ctf = ctxts[0][:].rearrange("p a b -> p (a b)")
                for cpart in range(2):
                    nc.vector.tensor_copy(st2[:], ctf[:, cpart * 1024:(cpart + 1) * 1024])
                    nc.sync.dma_start(dbg["ctxt"][:, cpart * 1024:(cpart + 1) * 1024], st2[:])
    nc.compile()
    return nc


def _prepare_inputs(hidden_states, qkv_w, qkv_b, dense_w):
    """Build per-core input maps (host-side slicing/packing, all bf16)."""
    bf16 = ml_dtypes.bfloat16
    x = np.ascontiguousarray(hidden_states, dtype=np.float32).reshape(BS, H)
    xtb = np.ascontiguousarray(x.T).astype(bf16)
    ident = np.eye(128, dtype=np.float32)
    qkv_w = np.asarray(qkv_w, dtype=np.float32)
    qkv_b = np.asarray(qkv_b, dtype=np.float32)
    dense_w = np.asarray(dense_w, dtype=np.float32)

    in_maps = []
    m = np.arange(128)
    jj, dd = m // 64, m % 64
    for c in range(NCORES):
        h = 2 * c + jj                      # head index per local dim m
        row_q = h * 192 + dd
        row_k = h * 192 + 64 + dd
        row_v = h * 192 + 128 + dd
        # w?[p, kk, m] = qkv_w[row(m), kk*128 + p]
        wq = np.ascontiguousarray(
            qkv_w[row_q, :].T.reshape(NKK, 128, 128).transpose(1, 0, 2)
        ).astype(bf16)
        wk = np.ascontiguousarray(
            qkv_w[row_k, :].T.reshape(NKK, 128, 128).transpose(1, 0, 2)
        ).astype(bf16)
        wv = np.ascontiguousarray(
            qkv_w[row_v, :].T.reshape(NKK, 128, 128).transpose(1, 0, 2)
        ).astype(bf16)
        # w2m[m, o] = dense_w[o, (2c + m//64)*64 + m%64]
        gcol = h * 64 + dd
        w2m = np.ascontiguousarray(dense_w[:, gcol].T).astype(bf16)
        qb = np.ascontiguousarray(qkv_b[row_q].reshape(128, 1),
                                  dtype=np.float32)
        in_maps.append({
            "xtb": xtb, "wq": wq, "wk": wk, "wv": wv, "w2m": w2m,
            "qbias": qb, "ident": ident,
        })
    return in_maps


def _reference_numpy(hidden_states, attention_mask, qkv_w, qkv_b, dense_w,
                     dense_b):
    """Exact fallback for non-all-ones masks (never hit with spec inputs)."""
    x = np.asarray(hidden_states, dtype=np.float64)
    mask = np.asarray(attention_mask, dtype=np.float64)
    mixed = x @ np.asarray(qkv_w, np.float64).T + np.asarray(qkv_b, np.float64)
    mixed = mixed.reshape(B, S, NH, 3 * HD).transpose(0, 2, 1, 3)
    q, k, v = np.split(mixed, 3, axis=-1)
    scores = np.einsum("bhqd,bhkd->bhqk", q, k) / np.sqrt(HD)
    scores = scores * mask - 10000.0 * (1.0 - mask)
    scores -= scores.max(axis=-1, keepdims=True)
    probs = np.exp(scores)
    probs /= probs.sum(axis=-1, keepdims=True)
    cx = np.einsum("bhqk,bhkd->bhqd", probs, v)
    cx = cx.transpose(0, 2, 1, 3).reshape(B, S, H)
    o = cx @ np.asarray(dense_w, np.float64).T + np.asarray(dense_b, np.float64)
    return o.astype(np.float32)


def _run(inputs, trace=False):
    from concourse.bass_utils import run_bass_kernel_spmd
    if "nc" not in _CACHE:
        _CACHE["nc"] = _build_program()
    nc = _CACHE["nc"]
    in_maps = _prepare_inputs(inputs["hidden_states"], inputs["qkv_w"],
                              inputs["qkv_b"], inputs["dense_w"])
    res = run_bass_kernel_spmd(nc, in_maps, core_ids=list(range(NCORES)),
                               trace=trace)
    partials = np.stack([r["out"] for r in res.results], axis=0)
    full = partials.sum(axis=0, dtype=np.float64)
    qkv_b = np.asarray(inputs["qkv_b"], dtype=np.float64)
    dense_w = np.asarray(inputs["dense_w"], dtype=np.float64)
    g = np.arange(H)
    bv = qkv_b[(g // HD) * 192 + 128 + (g % HD)]
    full += bv @ dense_w.T + np.asarray(inputs["dense_b"], dtype=np.float64)
    return full.astype(np.float32).reshape(B, S, H), res


def kernel(hidden_states, attention_mask, qkv_w, qkv_b, dense_w, dense_b):
    hidden_states = np.asarray(hidden_states)
    attention_mask = np.asarray(attention_mask)
    qkv_w = np.asarray(qkv_w)
    qkv_b = np.asarray(qkv_b)
    dense_w = np.asarray(dense_w)
    dense_b = np.asarray(dense_b)
    if not np.all(attention_mask == 1.0):
        return _reference_numpy(hidden_states, attention_mask, qkv_w, qkv_b,
                                dense_w, dense_b)
    out, _ = _run({
        "hidden_states": hidden_states, "qkv_w": qkv_w, "qkv_b": qkv_b,
        "dense_w": dense_w, "dense_b": dense_b,
    }, trace=bool(int(os.environ.get("KERNEL_TRACE", "0"))))
    return out


# revision 52
# speedup vs baseline: 1.1695x; 1.0006x over previous
"""Trainium2 Bass kernel for nn_Attention: 16-head attention, B=2, S=2048, H=1024.

Megatron-style tensor parallel over heads: 8 cores x 2 heads. Host sums the 8
partial dense outputs (all-reduce-after-dense recipe) and applies the bias
terms that commute out of the kernel.

Per-core dataflow (all matmul inputs bf16, fp32 PSUM accumulation):
  - q,k computed in [dim, token] layout (moving = x^T blocks, ap=512).
  - v computed directly in [token, dim] layout (stationary = x^T chunk,
    moving = v-weights), so no PE transposes are needed for v.
  - scores^T: PSUM [128 keys, 1024] holds two key-chunks x 512 queries; exp
    runs as one [128,1024] instruction, split between the Act engine (Exp
    activation, scale=1/8) and the DVE (pow with constant base e^{1/8}).
  - ctx accumulated in [token, dim] orientation: stationary = probs chunk,
    moving = v chunk with a ones column appended (65th column accumulates the
    softmax denominator for free).
  - late normalization (DVE reciprocal + per-partition scale), PE transpose of
    the normalized ctx to [dim, token], dense with moving = dense weights.
  - engine split: PE matmuls; Act = exp + q/k psum->sbuf (q-bias fused);
    DVE = exp + normalize + reciprocal; Pool = v/ctxT/dense-out copies + DMA
    queues for the streamed x^T blocks and output tiles.
  - bias handling: k-bias is softmax-invariant (dropped), v-bias and dense
    bias are added on the host, q-bias is fused into the q PSUM->SBUF copy.
"""
import math
import os

import numpy as np
import ml_dtypes

B, S, H, NH = 2, 2048, 1024, 16
HD = H // NH             # 64
BS = B * S               # 4096
NCORES = 8
NKK = H // 128           # 8 contraction chunks
NBLK = BS // 512         # 8 token blocks of 512
NQB = S // 512           # 4 query blocks per batch
NKC = S // 128           # 16 key chunks per batch
NPAIR = NKC // 2         # 8 key-chunk pairs per query block
NTC = BS // 128          # 32 token chunks of 128

_CACHE = {}

EXP_BASE = float(np.exp(0.125))  # e^{1/8}; (e^{1/8})^s == exp(s/8)


def _build_program():
    import concourse.mybir as mybir
    import concourse.tile as tile
    from concourse import bacc

    F32 = mybir.dt.float32
    F32R = mybir.dt.float32r
    BF16 = mybir.dt.bfloat16
    Act = mybir.ActivationFunctionType
    Alu = mybir.AluOpType

    nc = bacc.Bacc("TRN2", target_bir_lowering=False, debug=False,
                   num_devices=NCORES)
    xtb = nc.dram_tensor("xtb", [H, BS], BF16, kind="ExternalInput").ap()
    wq = nc.dram_tensor("wq", [128, NKK, 128], BF16, kind="ExternalInput").ap()
    wk = nc.dram_tensor("wk", [128, NKK, 128], BF16, kind="ExternalInput").ap()
    wv = nc.dram_tensor("wv", [128, NKK, 128], BF16, kind="ExternalInput").ap()
    w2m = nc.dram_tensor("w2m", [128, H], BF16, kind="ExternalInput").ap()
    qbias = nc.dram_tensor("qbias", [128, 1], F32, kind="ExternalInput").ap()
    ident = nc.dram_tensor("ident", [128, 128], F32R, kind="ExternalInput").ap()
    out = nc.dram_tensor("out", [BS, H], F32, kind="ExternalOutput").ap()
    dbg = {}
    if os.environ.get("KDBG"):
        dbg["qsb"] = nc.dram_tensor("dbg_qsb", [128, BS], F32, kind="ExternalOutput").ap()
        dbg["ksb"] = nc.dram_tensor("dbg_ksb", [128, BS], F32, kind="ExternalOutput").ap()
        dbg["vsb"] = nc.dram_tensor("dbg_vsb", [128, NTC * 130], F32, kind="ExternalOutput").ap()
        dbg["pt"] = nc.dram_tensor("dbg_pt", [128, 1024], F32, kind="ExternalOutput").ap()
        dbg["ctxa"] = nc.dram_tensor("dbg_ctxa", [128, NQB * 65], F32, kind="ExternalOutput").ap()
        dbg["rec"] = nc.dram_tensor("dbg_rec", [128, NQB], F32, kind="ExternalOutput").ap()
        dbg["ctxt"] = nc.dram_tensor("dbg_ctxt", [128, 16 * 128], F32, kind="ExternalOutput").ap()

    with tile.TileContext(nc) as tc, nc.allow_low_precision(reason="bf16"):
        from contextlib import ExitStack
        with ExitStack() as ctx:
            consts = ctx.enter_context(tc.tile_pool(name="consts", bufs=1))
            persist = ctx.enter_context(tc.tile_pool(name="persist", bufs=1))
            xtp = ctx.enter_context(tc.tile_pool(name="xtp", bufs=8))
            probs = ctx.enter_context(tc.tile_pool(name="probs", bufs=10))
            recipp = ctx.enter_context(tc.tile_pool(name="recipp", bufs=4))
            ctxnp = ctx.enter_context(tc.tile_pool(name="ctxnp", bufs=12))
            ctxtp = ctx.enter_context(tc.tile_pool(name="ctxtp", bufs=2))
            ostage = ctx.enter_context(tc.tile_pool(name="ostage", bufs=4))
            ps_sc = ctx.enter_context(
                tc.tile_pool(name="ps_sc", bufs=2, space="PSUM"))
            ps_ctx = ctx.enter_context(
                tc.tile_pool(name="ps_ctx", bufs=1, space="PSUM"))
            ps_qd = ctx.enter_context(
                tc.tile_pool(name="ps_qd", bufs=2, space="PSUM"))
            ps_ms = ctx.enter_context(
                tc.tile_pool(name="ps_ms", bufs=1, space="PSUM"))

            # ---- constants ----
            # wq first, then the first x^T block, so PE can start ASAP;
            # everything else queues behind on the SP DGE.
            wqsb = consts.tile([128, NKK, 128], BF16, name="wqsb")
            nc.sync.dma_start(wqsb[:, 0:NKK // 2, :], wq[:, 0:NKK // 2, :])
            xt_t0 = xtp.tile([128, NKK, 512], BF16, name="xt")
            src0 = xtb[:, 0:512].rearrange("(k p) t -> p k t", p=128)
            for kk in range(NKK // 2):
                nc.sync.dma_start(xt_t0[:, kk, :], src0[:, kk, :])
            nc.sync.dma_start(wqsb[:, NKK // 2:NKK, :], wq[:, NKK // 2:NKK, :])
            qbsb = consts.tile([128, 1], F32, name="qbsb")
            nc.sync.dma_start(qbsb[:], qbias)
            for kk in range(NKK // 2, NKK):
                nc.sync.dma_start(xt_t0[:, kk, :], src0[:, kk, :])
            wksb = consts.tile([128, NKK, 128], BF16, name="wksb")
            nc.sync.dma_start(wksb[:], wk)
            warm = consts.tile([1, 1], F32, name="warm")
            nc.scalar.activation(warm[0:1, 0:1], qbsb[0:1, 0:1], Act.Exp)

            # ---- persistent q/k/v ----
            qsb = persist.tile([128, BS], BF16, name="qsb")
            ksb = persist.tile([128, BS], BF16, name="ksb")
            # v layout: [token-part, chunk, 2*(64+1)]; cols 64 and 129 hold
            # the ones column that accumulates the softmax denominator.
            vsb = persist.tile([128, NTC, 130], BF16, name="vsb")
            nc.gpsimd.memset(vsb[:], 1.0)

            late_consts = {}

            def emit_late_consts():
                wvsb = consts.tile([128, NKK, 128], BF16, name="wvsb")
                nc.sync.dma_start(wvsb[:], wv)
                w2sb = consts.tile([128, H], BF16, name="w2sb")
                nc.sync.dma_start(w2sb[:], w2m)
                idsb = consts.tile([128, 128], F32R, name="idsb")
                nc.sync.dma_start(idsb[:], ident)
                late_consts.update(wvsb=wvsb, w2sb=w2sb, idsb=idsb)

            # ---- phase A: qkv projection for one 512-token block ----
            # Emitted as a list of small closures ("groups", ~0.6us of PE
            # work each) so blocks 4-7 can interleave into batch-0 attention
            # pairs as exp-independent PE filler.
            xts = {}

            def dma_block(n):
                if n == 0:
                    xts[n] = xt_t0
                    return
                xt_t = xtp.tile([128, NKK, 512], BF16, name="xt")
                src = xtb[:, n * 512:(n + 1) * 512].rearrange(
                    "(k p) t -> p k t", p=128)
                nc.sync.dma_start(xt_t[:], src)
                xts[n] = xt_t

            def qkv_groups(n):
                state = {}

                def g_start():
                    xt_t = state["xt"] = xts[n]
                    qps = ps_qd.tile([128, 512], F32, name="qps", tag="qd")
                    state["qps"] = qps
                    for kk in range(NKK // 2):
                        nc.tensor.matmul(qps[:], wqsb[:, kk, :], xt_t[:, kk, :],
                                         start=(kk == 0), stop=False)

                def g_q2():
                    xt_t, qps = state["xt"], state["qps"]
                    for kk in range(NKK // 2, NKK):
                        nc.tensor.matmul(qps[:], wqsb[:, kk, :], xt_t[:, kk, :],
                                         start=False, stop=(kk == NKK - 1))
                    if n < NBLK // 2:
                        nc.scalar.activation(qsb[:, n * 512:(n + 1) * 512],
                                             qps[:], Act.Identity,
                                             bias=qbsb[:, 0:1])
                    else:
                        nc.vector.tensor_scalar_add(
                            qsb[:, n * 512:(n + 1) * 512], qps[:],
                            qbsb[:, 0:1])

                def g_k1():
                    xt_t = state["xt"]
                    kps = ps_qd.tile([128, 512], F32, name="kps", tag="qd")
                    state["kps"] = kps
                    for kk in range(NKK // 2):
                        nc.tensor.matmul(kps[:], wksb[:, kk, :], xt_t[:, kk, :],
                                         start=(kk == 0), stop=False)

                def g_k2():
                    xt_t, kps = state["xt"], state["kps"]
                    for kk in range(NKK // 2, NKK):
                        nc.tensor.matmul(kps[:], wksb[:, kk, :], xt_t[:, kk, :],
                                         start=False, stop=(kk == NKK - 1))
                    if n < NBLK // 2:
                        nc.scalar.activation(ksb[:, n * 512:(n + 1) * 512],
                                             kps[:], Act.Identity)
                    else:
                        nc.vector.tensor_copy(
                            ksb[:, n * 512:(n + 1) * 512], kps[:])

                def g_v(t4):
                    xt_t = state["xt"]
                    if t4 == 0:
                        state["vps"] = ps_qd.tile([128, 4, 128], F32,
                                                  name="vps", tag="qd")
                    vps = state["vps"]
                    for kk in range(NKK):
                        nc.tensor.matmul(
                            vps[:, t4, :],
                            xt_t[:, kk, t4 * 128:(t4 + 1) * 128],
                            late_consts["wvsb"][:, kk, :],
                            start=(kk == 0 and t4 == 0),
                            stop=(kk == NKK - 1 and t4 == 3))
                    if t4 == 3:
                        for u4 in range(4):
                            g = n * 4 + u4
                            nc.vector.tensor_copy(
                                vsb[:, g, :].rearrange("p (j w) -> p j w",
                                                       w=65)[:, :, 0:64],
                                vps[:, u4, :].rearrange("p (j w) -> p j w",
                                                        w=64))

                return [g_start, g_q2, g_k1, g_k2,
                        lambda: g_v(0), lambda: g_v(1),
                        lambda: g_v(2), lambda: g_v(3)]

            def emit_qkv_block(n):
                dma_block(n)
                for g in qkv_groups(n):
                    g()

            def dbg_dump(name, ap_src, cols):
                if not dbg:
                    return
                st = ostage.tile([128, H], F32, name="dbgst")
                nc.vector.tensor_copy(st[:, 0:cols], ap_src)
                nc.sync.dma_start(dbg[name][:, 0:cols], st[:, 0:cols])

            # ---- phase B helpers ----
            def emit_norm(pend):
                """Reciprocal of the denominators + unnormalized ctx to
                SBUF (normalization folds into the dense-output scale)."""
                b, j, qb, ctxa, ctxt = pend
                den = recipp.tile([128, NQB], F32, name="den", tag="den")
                nc.vector.tensor_copy(den[:], ctxa[:, :, 64])
                rec = recipp.tile([128, NQB], F32, name="rec", tag="rec")
                nc.vector.reciprocal_approx_fast(rec[:], den[:])
                if dbg and (b, j, qb) == (0, 0, 0):
                    dbg_dump("rec", rec[:], NQB)
                cns = [rec]
                for qc in range(NQB):
                    cn = ctxnp.tile([128, 64], F32R, name="cn")
                    nc.vector.tensor_scalar_mul(cn[:], ctxa[:, qc, 0:64],
                                                rec[:, qc:qc + 1])
                    cns.append(cn)
                return cns

            def emit_tp(pend, cns):
                """Transpose normalized ctx to [dim, token] (PE + Pool)."""
                b, j, qb, ctxa, ctxt = pend
                tp = ps_ms.tile([128, NQB, 128], F32R, name="tp", tag="ms")
                for qc in range(NQB):
                    nc.tensor.matmul(tp[0:64, qc, :],
                                     cns[1 + qc][:],
                                     late_consts["idsb"][:],
                                     is_transpose=True,
                                     start=(qc == 0), stop=(qc == NQB - 1))
                for qc in range(NQB):
                    t = qb * 4 + qc
                    nc.vector.tensor_copy(
                        ctxt[j * 64:(j + 1) * 64, t, :],
                        tp[0:64, qc, :])

            def emit_dense_qc(pend, qc, rec, last=False):
                """Dense partial for one 128-token chunk of pend (j==1);
                the softmax normalization is applied here as a per-token
                scale on the PSUM->SBUF drain."""
                b, j, qb, ctxa, ctxt = pend
                t = qb * 4 + qc
                od = ostage.tile([128, H], F32, name="od")
                for nb in range(2):
                    dp = ps_qd.tile([128, 512], F32, name="dp", tag="qd")
                    nc.tensor.matmul(
                        dp[:], ctxt[:, t, :],
                        late_consts["w2sb"][:, nb * 512:(nb + 1) * 512],
                        start=True, stop=True)
                    nc.vector.tensor_copy(
                        od[:, nb * 512:(nb + 1) * 512], dp[:])
                row0 = b * S + t * 128
                nc.sync.dma_start(out[row0:row0 + 128, :], od[:])

            dense_jobs = []    # (pend, qc) waiting for a dense slot
            fill_jobs = []     # qkv groups to interleave as PE filler

            def emit_attention_qb(b, j, qb, ctxt, pend, exp_pat):
                """One 512-query block of head j, batch b. Returns new pend."""
                ctxa = ps_ctx.tile([128, NQB, 65], F32, name="ctxa")
                pts = {}
                for pr in range(NPAIR):
                    for _ in range(fill_rate[0](pr)):
                        if fill_jobs:
                            fill_jobs.pop(0)()
                    sp = ps_sc.tile([128, 1024], F32, name="sp")
                    for hf in range(2):
                        kc = pr * 2 + hf
                        nc.tensor.matmul(
                            sp[:, hf * 512:(hf + 1) * 512],
                            ksb[j * 64:(j + 1) * 64,
                                b * S + kc * 128:b * S + (kc + 1) * 128],
                            qsb[j * 64:(j + 1) * 64,
                                b * S + qb * 512:b * S + (qb + 1) * 512],
                            start=True, stop=True)
                    # exp runs on Act only: the DVE datapath has no exp
                    # and GPSIMD cannot read PSUM on TRN2.
                    pt = probs.tile([128, 1024], BF16, name="pt")
                    nc.scalar.activation(pt[:], sp[:], Act.Exp, scale=0.125)
                    pts[pr] = pt
                    if dbg and (b, j, qb, pr) == (0, 0, 0, 0):
                        dbg_dump("pt", pt[:], 1024)
                    if pr == 2 and pend is not None:
                        emit_tp(pend[:5], pend[5])
                    if pr in (4, 6) and dense_jobs:
                        dpend, qc, drec = dense_jobs.pop(0)
                        emit_dense_qc(dpend, qc, drec)
                    if pr >= 2:
                        emit_ctx(b, j, pr - 2, ctxa, pts.pop(pr - 2))
                emit_ctx(b, j, NPAIR - 2, ctxa, pts.pop(NPAIR - 2))
                emit_ctx(b, j, NPAIR - 1, ctxa, pts.pop(NPAIR - 1))
                cur = (b, j, qb, ctxa, ctxt)
                if dbg and (b, j, qb) == (0, 0, 0):
                    dbg_dump("ctxa", ctxa[:].rearrange("p a b -> p (a b)"), NQB * 65)
                cns = emit_norm(cur)
                if j == 1:
                    for qc in range(NQB):
                        dense_jobs.append((cur, qc, cns[0]))
                return cur + (cns,)

            def emit_ctx(b, j, pr, ctxa, pt):
                # a start=True matmul zeroes the whole 2KB PSUM bank, so the
                # four qc sub-accumulators chain into ONE group: only the
                # first matmul starts it, only the last stops it.
                for hf in range(2):
                    kc = pr * 2 + hf
                    for qc in range(NQB):
                        nc.tensor.matmul(
                            ctxa[:, qc, :],
                            pt[:, hf * 512 + qc * 128:hf * 512 + (qc + 1) * 128],
                            vsb[:, b * 16 + kc, j * 65:(j + 1) * 65],
                            start=(kc == 0 and qc == 0),
                            stop=(kc == NKC - 1 and qc == NQB - 1))

            # ---- emission schedule ----
            # Blocks 0-3 (batch 0) up front; blocks 4-7 interleave into
            # batch-0 attention as pair-level PE filler via fill_jobs.
            emit_late_consts()
            emit_qkv_block(0)
            # blocks 1-7: DMAs issued now (transfers run ahead of PE); the
            # compute groups interleave into attention as pair-level filler.
            # Attention on (batch 0, qb 0) only needs k/v blocks progressively
            # (scores for key-chunk kc reads k-block kc//4), so it starts
            # right after block 0.
            for n in range(1, NBLK):
                dma_block(n)
                fill_jobs.extend(qkv_groups(n))
            PAT = ["D", "A"] * 4
            fill_rate = [lambda pr: 0]
            region = ["a0"]
            pend = None
            ctxts = {0: ctxtp.tile([128, S // 128, 128], BF16, name="ctxt0")}
            # j interleaved at qb granularity so dense work (which becomes
            # ready only after a j==1 block) spreads across every qb slot.
            QB0_RATE = [8, 6, 4, 2, 2, 2, 0, 0]
            for qb in range(NQB):
                for j in range(2):
                    if (j, qb) == (0, 0):
                        fill_rate[0] = lambda pr: QB0_RATE[pr]
                    else:
                        fill_rate[0] = lambda pr: 1 if pr in (1, 3, 5, 7) else 0
                    pend = emit_attention_qb(0, j, qb, ctxts[0], pend, PAT)
            if dbg:
                for cpart in range(BS // 512):
                    dbg_dump("qsb", qsb[:, cpart * 512:(cpart + 1) * 512], 512)
                # overwrite-style: dump full via multiple stages
            if dbg:
                st = ostage.tile([128, H], F32, name="dbgq2")
                for cpart in range(BS // 1024):
                    nc.vector.tensor_copy(st[:], qsb[:, cpart * 1024:(cpart + 1) * 1024])
                    nc.sync.dma_start(dbg["qsb"][:, cpart * 1024:(cpart + 1) * 1024], st[:])
                    nc.vector.tensor_copy(st[:], ksb[:, cpart * 1024:(cpart + 1) * 1024])
                    nc.sync.dma_start(dbg["ksb"][:, cpart * 1024:(cpart + 1) * 1024], st[:])
                vflat = vsb[:].rearrange("p a b -> p (a b)")
                for cpart in range(5):
                    w = min(1024, NTC * 130 - cpart * 1024)
                    nc.vector.tensor_copy(st[:, 0:w], vflat[:, cpart * 1024:cpart * 1024 + w])
                    nc.sync.dma_start(dbg["vsb"][:, cpart * 1024:cpart * 1024 + w], st[:, 0:w])
            ctxts[1] = ctxtp.tile([128, S // 128, 128], BF16, name="ctxt1")
            region[0] = "a1"
            fill_rate[0] = lambda pr: 1
            pend = emit_attention_qb(1, 0, 0, ctxts[1], pend, PAT)
            fill_rate[0] = lambda pr: 0
            while fill_jobs:
                fill_jobs.pop(0)()
            for qb in range(NQB):
                for j in range(2):
                    if (j, qb) == (0, 0):
                        continue
                    pend = emit_attention_qb(1, j, qb, ctxts[1], pend, PAT)
            while fill_jobs:
                fill_jobs.pop(0)()
            emit_tp(pend[:5], pend[5])
            while dense_jobs:
                dpend, qc, drec = dense_jobs.pop(0)
                emit_dense_qc(dpend, qc, drec, last=True)
            if dbg:
                st2 = ostage.tile([128, H], F32, name="dbgct")
                ctf = ctxts[0][:].rearrange("p a b -> p (a b)")
                for cpart in range(2):
                    nc.vector.tensor_copy(st2[:], ctf[:, cpart * 1024:(cpart + 1) * 1024])
                    nc.sync.dma_start(dbg["ctxt"][:, cpart * 1024:(cpart + 1) * 1024], st2[:])
    nc.compile()
    return nc


def _prepare_inputs(hidden_states, qkv_w, qkv_b, dense_w):
    """Build per-core input maps (host-side slicing/packing, all bf16)."""
    bf16 = ml_dtypes.bfloat16
    x = np.ascontiguousarray(hidden_states, dtype=np.float32).reshape(BS, H)
    xtb = np.ascontiguousarray(x.T).astype(bf16)
    ident = np.eye(128, dtype=np.float32)
    qkv_w = np.asarray(qkv_w, dtype=np.float32)
    qkv_b = np.asarray(qkv_b, dtype=np.float32)
    dense_w = np.asarray(dense_w, dtype=np.float32)

    in_maps = []
    m = np.arange(128)
    jj, dd = m // 64, m % 64
    for c in range(NCORES):
        h = 2 * c + jj                      # head index per local dim m
        row_q = h * 192 + dd
        row_k = h * 192 + 64 + dd
        row_v = h * 192 + 128 + dd
        # w?[p, kk, m] = qkv_w[row(m), kk*128 + p]
        wq = np.ascontiguousarray(
            qkv_w[row_q, :].T.reshape(NKK, 128, 128).transpose(1, 0, 2)
        ).astype(bf16)
        wk = np.ascontiguousarray(
            qkv_w[row_k, :].T.reshape(NKK, 128, 128).transpose(1, 0, 2)
        ).astype(bf16)
        wv = np.ascontiguousarray(
            qkv_w[row_v, :].T.reshape(NKK, 128, 128).transpose(1, 0, 2)
        ).astype(bf16)
        # w2m[m, o] = dense_w[o, (2c + m//64)*64 + m%64]
        gcol = h * 64 + dd
        w2m = np.ascontiguousarray(dense_w[:, gcol].T).astype(bf16)
        qb = np.ascontiguousarray(qkv_b[row_q].reshape(128, 1),
                                  dtype=np.float32)
        in_maps.append({
            "xtb": xtb, "wq": wq, "wk": wk, "wv": wv, "w2m": w2m,
            "qbias": qb, "ident": ident,
        })
    return in_maps


def _reference_numpy(hidden_states, attention_mask, qkv_w, qkv_b, dense_w,
                     dense_b):
    """Exact fallback for non-all-ones masks (never hit with spec inputs)."""
    x = np.asarray(hidden_states, dtype=np.float64)
    mask = np.asarray(attention_mask, dtype=np.float64)
    mixed = x @ np.asarray(qkv_w, np.float64).T + np.asarray(qkv_b, np.float64)
    mixed = mixed.reshape(B, S, NH, 3 * HD).transpose(0, 2, 1, 3)
    q, k, v = np.split(mixed, 3, axis=-1)
    scores = np.einsum("bhqd,bhkd->bhqk", q, k) / np.sqrt(HD)
    scores = scores * mask - 10000.0 * (1.0 - mask)
    scores -= scores.max(axis=-1, keepdims=True)
    probs = np.exp(scores)
    probs /= probs.sum(axis=-1, keepdims=True)
    cx = np.einsum("bhqk,bhkd->bhqd", probs, v)
    cx = cx.transpose(0, 2, 1, 3).reshape(B, S, H)
    o = cx @ np.asarray(dense_w, np.float64).T + np.asarray(dense_b, np.float64)
    return o.astype(np.float32)


def _run(inputs, trace=False):
    from concourse.bass_utils import run_bass_kernel_spmd
    if "nc" not in _CACHE:
        _CACHE["nc"] = _build_program()
    nc = _CACHE["nc"]
    in_maps = _prepare_inputs(inputs["hidden_states"], inputs["qkv_w"],
                              inputs["qkv_b"], inputs["dense_w"])
    res = run_bass_kernel_spmd(nc, in_maps, core_ids=list(range(NCORES)),
                               trace=trace)
    partials = np.stack([r["out"] for r in res.results], axis=0)
    full = partials.sum(axis=0, dtype=np.float64)
    qkv_b = np.asarray(inputs["qkv_b"], dtype=np.float64)
    dense_w = np.asarray(inputs["dense_w"], dtype=np.float64)
    g = np.arange(H)
    bv = qkv_b[(g // HD) * 192 + 128 + (g % HD)]
    full += bv @ dense_w.T + np.asarray(inputs["dense_b"], dtype=np.float64)
    return full.astype(np.float32).reshape(B, S, H), res


def kernel(hidden_states, attention_mask, qkv_w, qkv_b, dense_w, dense_b):
    hidden_states = np.asarray(hidden_states)
    attention_mask = np.asarray(attention_mask)
    qkv_w = np.asarray(qkv_w)
    qkv_b = np.asarray(qkv_b)
    dense_w = np.asarray(dense_w)
    dense_b = np.asarray(dense_b)
    if not np.all(attention_mask == 1.0):
        return _reference_numpy(hidden_states, attention_mask, qkv_w, qkv_b,
                                dense_w, dense_b)
    out, _ = _run({
        "hidden_states": hidden_states, "qkv_w": qkv_w, "qkv_b": qkv_b,
        "dense_w": dense_w, "dense_b": dense_b,
    }, trace=bool(int(os.environ.get("KERNEL_TRACE", "0"))))
    return out
